# revision 86
# baseline (speedup 1.0000x reference)
"""Trainium2 Bass kernel for causal+padded multi-head attention.

Problem: B=2, N=2048, D=1024, H=16 heads (DK=64), fp32 I/O.
  out = softmax(mask(x Wq^T (x Wk^T)^T) / sqrt(DK)) (x Wv^T) Wout^T + b_out

Sharding (8 cores): core c handles batch b=c//4 and heads [4*(c%4), 4*(c%4)+4).
Each core computes a partial output [N, D] (its 4 heads' contribution through
the output projection); the host sums the 4 partials per batch and adds b_out.

On-device layout (per core):
  xT   [1024, 2048]  (host-pretransposed x[b])
  QT/KT stored transposed [dk, n] as head-pair tiles [128, 2048]
  V    stored natural as [128(keys), 16 blocks, 4 heads, 65] with a ones
       column appended (col 64) so P@V' also yields the softmax denominator.
  S^T  computed per (head-pair, q-tile 512, key-block 128) as [128, 2, 512]
       in PSUM: matmul(lhsT=KT slice [64,128], rhs=QT slice [64,512]); the
       two heads sit at base partitions 0/64 so their matmuls row-tile and
       run concurrently on the PE.
       Causal masking = additive -30000 on PSUM (DVE); padding mask is a
       per-key bias fused into the exp; one exp(0.125*s + bias) on ScalarE
       writes P^T straight to SBUF as bf16.
  ctx'^T [65, 512] accumulated in PSUM over key blocks:
       matmul(lhsT=V' [128,65], rhs=P^T [128,512]); PV matmuls are emitted
       one unit behind their exps so the in-order PE never waits on ScalarE
       (idle slivers would re-throttle the HAM clock gate to 1.2 GHz).
  Normalization: r = recip(rowsum) on the [1,512] denominator row (DVE),
       partition-broadcast to [64,512] (GpSimd), one DVE multiply;
       software-pipelined one unit behind the PV.
  Out projection: matmul(lhsT=ctxT [128,128], rhs=WoutT [128,512]) acc over
       the two head-pair chunks.

All matmul operands are bf16 (pre-rounded on host for the inputs; on-device
casts for intermediates); accumulation is fp32 in PSUM, and the softmax /
masking / normalization arithmetic is fp32. float32r was measured on this
hardware at 2 cycles/row with a serial (non-FWL) weight load that starves
the HAM activity monitor - bf16 is ~2.7x faster per matmul in practice.
"""

import math
import os

import numpy as np

B, N, D, H = 2, 2048, 1024, 16
DK = D // H  # 64
NCORES = 8
HEADS_PER_CORE = 4
QTILE = 512
KBLK = 128
NEG = -30000.0
NEGB = -3750.0  # pad bias applied after the 0.125 scale inside exp
SCALE = 1.0 / math.sqrt(float(DK))  # 0.125

# Set by run() when tracing is enabled (test.py reads this).
LAST_RESULTS = None


def _build_program(kb_max: int, jpad_min: int):
    import concourse.tile as tile
    from concourse import bacc, mybir

    F32 = mybir.dt.float32
    F32R = mybir.dt.float32r
    BF16 = mybir.dt.bfloat16
    EXP = mybir.ActivationFunctionType.Exp
    ADD = mybir.AluOpType.add

    nc = bacc.Bacc(None)

    # weights arrive pre-arranged on the host into the on-chip layout so
    # every DMA is a contiguous [128, x] transfer (strided gathers stall the
    # critical first-projection path)
    xt_d = nc.dram_tensor("xt", [D, N], BF16, kind="ExternalInput")
    wq_d = nc.dram_tensor("wq", [128, 2048], BF16, kind="ExternalInput")
    wk_d = nc.dram_tensor("wk", [128, 2048], BF16, kind="ExternalInput")
    wv_d = nc.dram_tensor("wv", [128, 2048], BF16, kind="ExternalInput")
    wout_d = nc.dram_tensor("wout", [128, 2048], BF16, kind="ExternalInput")
    padb_d = nc.dram_tensor("padbias", [128, 16], F32, kind="ExternalInput")
    trimask_d = nc.dram_tensor("trimask", [128, 1792], BF16, kind="ExternalInput")
    ones_d = nc.dram_tensor("ones65", [128, 64], BF16, kind="ExternalInput")
    out_d = nc.dram_tensor("out", [N, D], BF16, kind="ExternalOutput")

    NB = N // KBLK  # 16 key/row blocks
    NQT = N // QTILE  # 4 q tiles

    with tile.TileContext(nc) as tc:
        with (
            tc.tile_pool(name="w", bufs=1) as w_pool,
            tc.tile_pool(name="big", bufs=1) as big_pool,
            tc.tile_pool(name="work", bufs=3) as work_pool,
            tc.tile_pool(name="ps_main", bufs=2, space="PSUM") as ps_main,
            tc.tile_pool(name="ps_ctx", bufs=2, space="PSUM") as ps_ctx,
        ):
            # ---- load inputs ----
            # Order matters: wq/wk then the xt chunks gate the first Q/K
            # projection, which gates the whole pipeline. Everything else
            # streams behind them.
            xt_cm = tc.tile_pool(name="xt", bufs=8)
            xt_pool = xt_cm.__enter__()
            wq_t = w_pool.tile([128, 8, 256], BF16, tag="wq")
            wk_t = w_pool.tile([128, 8, 256], BF16, tag="wk")
            wv_t = w_pool.tile([128, 8, 256], BF16, tag="wv")
            wo_t = w_pool.tile([128, 2, D], BF16, tag="wo")
            padb_t = w_pool.tile([128, 16], F32, tag="padb")
            trimask_t = w_pool.tile([128, 2, 896], BF16, tag="trimask")
            # The first q-tile of x streams per-chunk so the Q projection's
            # accumulation steps pipeline with DMA arrival. The critical
            # startup prefix (wq/wk/xt q-tile 0) is split across the sync
            # and gpsimd DMA queues — a single queue tops out well below
            # HBM bandwidth. Everything the startup doesn't need is gated
            # behind the first K cast.
            xt = [
                xt_pool.tile([128, N], BF16, tag="xt", name=f"xt{e}")
                for e in range(8)
            ]
            # interleave the weight-chunk and x-chunk transfers so the first
            # Q/K accumulation steps start on the first ~0.3MB instead of
            # waiting for whole tensors
            # prefix split across three DMA queues (sync/vector/gpsimd) —
            # one queue tops out at ~160GB/s, a third of per-core HBM
            for e in range(8):
                nc.sync.dma_start(wq_t[:, e, :], wq_d[:, e * 256:(e + 1) * 256])
                eng = nc.sync if e % 2 == 0 else nc.scalar
                eng.dma_start(
                    xt[e][:, 0:512], xt_d[e * 128:(e + 1) * 128, 0:512]
                )
                nc.gpsimd.dma_start(wk_t[:, e, :], wk_d[:, e * 256:(e + 1) * 256])
            nc.sync.dma_start(wv_t[:].rearrange("p e m -> p (e m)"), wv_d[:])
            nc.sync.dma_start(padb_t[:], padb_d[:])
            nc.gpsimd.dma_start(
                trimask_t[:], trimask_d[:].rearrange("p (h u) -> p h u", h=2)
            )

            # Warm the ScalarE activation table (exp set) during the input
            # DMA window so the first real exp doesn't eat the ~2.7us load.
            warm_in = w_pool.tile([1, 8], F32, tag="warm_in")
            warm_out = w_pool.tile([1, 8], F32, tag="warm_out")
            nc.gpsimd.memset(warm_in[:], 0.0)
            nc.scalar.activation(warm_out[:], warm_in[:], EXP)

            # V' tile: [keys 128, key-block 16, head 4, 128]; col 64 <- ones,
            # cols 65-127 <- 0 (padding to 128 weight columns keeps FWL on for
            # the PV matmuls; PSUM rows 65-127 of ctx' are dead).
            v4 = big_pool.tile([128, NB, 4, 128], BF16, tag="v4")
            nc.vector.memset(v4[:, :, :, 65:128], 0.0)
            nc.sync.dma_start(
                v4[:, :, :, 64:65],
                ones_d[:].rearrange("p (b h o) -> p b h o", h=4, o=1),
            )

            qt_pair = [big_pool.tile([128, N], BF16, tag=f"qt{p}", name=f"qt{p}") for p in range(2)]
            kt_pair = [big_pool.tile([128, N], BF16, tag=f"kt{p}", name=f"kt{p}") for p in range(2)]
            ctx_pair = [big_pool.tile([128, N], BF16, tag=f"ctx{p}", name=f"ctx{p}") for p in range(2)]

            pt_cm = tc.tile_pool(name="pt", bufs=26)
            pt_pool = pt_cm.__enter__()

            # ---- projection tasks, riffled into the attention stream ----
            # QT/KT: [dk(128 = 2 heads), n] = (W.T chunk)^T @ xT
            def xt_slice(e, c0, c1):
                return xt[e][:, c0:c1]

            def emit_qk_proj(w_t, dst, pair, nq):
                ps = ps_main.tile([128, 2, 512], F32, tag="blk", name="blk")[:, 0, :]
                for e in range(8):
                    nc.tensor.matmul(
                        ps[:],
                        w_t[:, e, pair * 128:(pair + 1) * 128],
                        xt_slice(e, nq * 512, (nq + 1) * 512),
                        start=(e == 0),
                        stop=(e == 7),
                    )
                nc.vector.tensor_copy(dst[pair][:, nq * 512:(nq + 1) * 512], ps[:])

            def emit_qk_fused(pair, nq):
                # prologue: Q and K accumulate per x-chunk in lockstep so
                # both finish ~one matmul after the last x chunk arrives
                psq = ps_main.tile([128, 2, 512], F32, tag="blk", name="psq")[:, 0, :]
                psk = ps_main.tile([128, 2, 512], F32, tag="blk", name="psk")[:, 0, :]
                for e in range(8):
                    nc.tensor.matmul(
                        psq[:],
                        wq_t[:, e, pair * 128:(pair + 1) * 128],
                        xt_slice(e, nq * 512, (nq + 1) * 512),
                        start=(e == 0),
                        stop=(e == 7),
                    )
                    nc.tensor.matmul(
                        psk[:],
                        wk_t[:, e, pair * 128:(pair + 1) * 128],
                        xt_slice(e, nq * 512, (nq + 1) * 512),
                        start=(e == 0),
                        stop=(e == 7),
                    )
                nc.vector.tensor_copy(
                    kt_pair[pair][:, nq * 512:(nq + 1) * 512], psk[:]
                )
                nc.vector.tensor_copy(
                    qt_pair[pair][:, nq * 512:(nq + 1) * 512], psq[:]
                )

            # V natural: [n-block, 4*64] = xT-chunk^T @ WvT-chunk
            def emit_v_proj(nb):
                ps = ps_main.tile([128, 2, 512], F32, tag="blk", name="blk")[:, 0, 0:256]
                for e in range(8):
                    nc.tensor.matmul(
                        ps[:],
                        xt_slice(e, nb * 128, (nb + 1) * 128),
                        wv_t[:, e, :],
                        start=(e == 0),
                        stop=(e == 7),
                    )
                nc.vector.tensor_copy(
                    v4[:, nb, :, 0:64],
                    ps[:].rearrange("p (h d) -> p h d", h=4),
                )

            # Unit order: q0, q2, q3, q1. Starting on q0 keeps the cold-clock
            # prologue to two projection groups; the exp-heavy q2/q3 stretch
            # sits mid-stream where V-block/outproj filler is plentiful; the
            # small-PV q1 tail keeps the post-last-exp drain short.
            # Each projection task carries a (unit, slot) deadline: its S^T
            # needs Q at slot 0 and K tile t by slot 4t; the riffled PV of
            # unit u-1 needs V block nb by slot nb of unit u. Tasks drain
            # lazily at ~1 per 2 chunk slots so filler PE work carries
            # forward into the exp-heavy units; deadlines force correctness.
            qt_order = [0, 2, 3, 1]
            units = [
                (pair, qt, min(4 * qt + 4, kb_max))
                for qt in qt_order
                for pair in range(2)
            ]

            def t_q(pair, r):
                return lambda: emit_qk_proj(wq_t, qt_pair, pair, r)

            def t_k(pair, r):
                return lambda: emit_qk_proj(wk_t, kt_pair, pair, r)

            def t_v(nb):
                return lambda: emit_v_proj(nb)

            tasks = []  # (unit, slot, thunk)
            k_done = [set(), set()]
            v_done = set()
            for ui, (pair, qt, nchunks) in enumerate(units):
                if ui == 0:
                    # fused Q+K prologue for the very first unit
                    k_done[pair].add(qt)
                    tasks.append((0, 0, lambda p=pair, r=qt: emit_qk_fused(p, r)))
                else:
                    tasks.append((ui, 0, t_q(pair, qt)))
                for t in range(qt + 1):
                    if t not in k_done[pair]:
                        k_done[pair].add(t)
                        tasks.append((ui, 4 * t, t_k(pair, t)))
                if ui + 1 < len(units):
                    # V blocks consumed by this unit's PV, riffled in unit+1
                    for nb in range(nchunks):
                        if nb not in v_done:
                            v_done.add(nb)
                            tasks.append((ui + 1, nb, t_v(nb)))
            tasks.sort(key=lambda e: (e[0], e[1]))
            proj_fifo = list(tasks)
            filler_q = []  # deadline-free PE filler (outproj nb-blocks)

            def drain_due(ui, j):
                while proj_fifo and (proj_fifo[0][0], proj_fifo[0][1]) <= (ui, j):
                    proj_fifo.pop(0)[2]()

            def drain_lazy():
                if proj_fifo:
                    proj_fifo.pop(0)[2]()
                elif filler_q:
                    filler_q.pop(0)()

            # ---- attention, head pairs interleaved ----
            # A unit is (head-pair, q-tile). The two heads' S^T matmuls sit
            # at base partitions 0 / 64 (row groups 0-63 / 64-127), so they
            # execute concurrently on the PE and their weight loads overlap
            # the other head's matmul — no LDW bubble, HAM stays warm.
            # PV matmuls run one unit behind their exps so the in-order PE
            # never drains waiting on ScalarE.
            def emit_normalize(pair, hh, qt, ctx_ps):
                # the craw copy doubles as the PSUM release: it is the only
                # reader of ctx_ps rows 0-63, so the next unit's PV (same
                # single-buffered bank) can start as soon as it completes
                # instead of waiting for the whole normalize chain
                hp = slice(64 * hh, 64 * hh + 64)
                craw = work_pool.tile([64, 512], F32, tag="craw", name="craw")
                nc.vector.tensor_copy(craw[:], ctx_ps[0:64, :])
                rden = work_pool.tile([1, 512], F32, tag="rden", name="rden")
                nc.scalar.copy(rden[:], ctx_ps[64:65, :])
                rrec = work_pool.tile([1, 512], F32, tag="rrec", name="rrec")
                nc.vector.reciprocal_approx_fast(rrec[:], rden[:])
                rbr = work_pool.tile([64, 512], F32, tag="rbr", name="rbr")
                nc.gpsimd.partition_broadcast(rbr[:], rrec[:])
                nc.vector.tensor_mul(
                    ctx_pair[pair][hp, qt * 512:(qt + 1) * 512],
                    craw[0:64, :],
                    rbr[:],
                )

            def emit_st_exp(ui, pair, qt, nchunks, prev, self_ctx2=None):
                """S^T + mask + exp for both heads, with the previous unit's
                PV matmuls riffled in (they are long-ready and fill the PE
                slots where S^T would stall on the exp pipeline). Returns
                PV descriptors."""
                if prev is None:
                    ppv = []
                else:
                    ppair, pqt, pn, ppv, pctx2 = prev

                def rif(k):
                    # emit previous-unit PV chunks up to index k
                    while ppv and ppv[0][0] <= k:
                        jj, ptt, poff = ppv.pop(0)
                        for hh in range(2):
                            nc.tensor.matmul(
                                pctx2[hh][:, poff:],
                                v4[:, jj, 2 * ppair + hh, :],
                                ptt[:, hh, poff:],
                                start=(jj == 0),
                                stop=(jj == pn - 1),
                                skip_group_check=True,
                            )

                pv = []
                last_units = ui >= len(units) - 2
                for j in range(nchunks):
                    drain_due(ui, j)
                    rif(j)
                    # drain filler every other slot; every slot near the end
                    # so no PE work is left to trail the last exps
                    if last_units or j % 2 == 1:
                        drain_lazy()
                    d = j - 4 * qt
                    # exact-causal column trim (keep matmul N >= 256)
                    off = 128 * d if d >= 1 else 0
                    st_ps = ps_main.tile([128, 2, 512], F32, tag="blk", name="blk")
                    for hh in range(2):
                        hp = slice(64 * hh, 64 * hh + 64)
                        nc.tensor.matmul(
                            st_ps[:, hh, off:],
                            kt_pair[pair][hp, j * 128:(j + 1) * 128],
                            qt_pair[pair][hp, qt * 512 + off:(qt + 1) * 512],
                            start=True,
                            stop=True,
                        )
                    pt_t = pt_pool.tile([128, 2, 512], BF16, tag="pt")
                    kw = {}
                    if j >= jpad_min:  # per-key pad bias (same for both heads)
                        kw["bias"] = padb_t[:, j:j + 1]
                    nc.scalar.activation(
                        pt_t[:, :, off:], st_ps[:, :, off:], EXP, scale=SCALE, **kw
                    )
                    if d >= 0:
                        # causal mask as a post-exp 0/1 multiply; per-head 2D
                        # slices keep the DVE in its fast dense mode (a 3D
                        # strided AP drops it to 1x). With off = 128*d the
                        # masked triangle lies entirely in cols [off, off+128)
                        u0 = 384 - 128 * d + off
                        w = min(128, 512 - off)
                        for hh in range(2):
                            nc.vector.tensor_mul(
                                pt_t[:, hh, off:off + w],
                                pt_t[:, hh, off:off + w],
                                trimask_t[:, hh, u0:u0 + w],
                            )
                    pv.append((j, pt_t, off))
                    if self_ctx2 is not None and len(pv) >= 2:
                        # final unit: riffle its own PV one chunk behind the
                        # exps so the tail isn't a serial PV stream after
                        # the last exp
                        jj, ptt, poff = pv.pop(0)
                        for hh in range(2):
                            nc.tensor.matmul(
                                self_ctx2[hh][:, poff:],
                                v4[:, jj, 2 * pair + hh, :],
                                ptt[:, hh, poff:],
                                start=(jj == 0),
                                stop=(jj == nchunks - 1),
                                skip_group_check=True,
                            )
                rif(10 ** 9)
                return pv

            def emit_pv(pair, qt, nchunks, pv, ctx2):
                for j, pt_t, off in pv:
                    for hh in range(2):
                        nc.tensor.matmul(
                            ctx2[hh][:, off:],
                            v4[:, j, 2 * pair + hh, :],
                            pt_t[:, hh, off:],
                            start=(j == 0),
                            stop=(j == nchunks - 1),
                            skip_group_check=True,
                        )

            done_norms = {q: 0 for q in range(NQT)}
            d_emitted = set()

            def emit_outproj_nb(nb, split):
                # output projection for one 128-token block; `split` sends
                # the fc=1 PSUM->SBUF cast to ScalarE (tail drain, when
                # ScalarE has gone idle) instead of DVE.
                osb = work_pool.tile([128, D], BF16, tag="osb", name="osb")
                for fc in range(2):
                    ps = ps_main.tile(
                        [128, 2, 512], F32, tag="blk", name="blk"
                    )[:, 0, :]
                    for pr2 in range(2):
                        nc.tensor.matmul(
                            ps[:],
                            ctx_pair[pr2][:, nb * 128:(nb + 1) * 128],
                            wo_t[:, pr2, fc * 512:(fc + 1) * 512],
                            start=(pr2 == 0),
                            stop=(pr2 == 1),
                        )
                    if split and fc == 1:
                        nc.scalar.copy(osb[:, fc * 512:(fc + 1) * 512], ps[:])
                    else:
                        nc.vector.tensor_copy(
                            osb[:, fc * 512:(fc + 1) * 512], ps[:]
                        )
                if split:
                    # tail: split the write across two DMA queues (one queue
                    # tops out at ~140GB/s; gpsimd's queue is free by now)
                    nc.sync.dma_start(
                        out_d[nb * 128:(nb + 1) * 128, 0:512], osb[:, 0:512]
                    )
                    nc.gpsimd.dma_start(
                        out_d[nb * 128:(nb + 1) * 128, 512:D], osb[:, 512:D]
                    )
                else:
                    nc.sync.dma_start(out_d[nb * 128:(nb + 1) * 128, :], osb[:])

            def pop_norm(tail=False):
                npair, nqt, nctx2 = norm_q.pop(0)
                for hh in range(2):
                    emit_normalize(npair, hh, nqt, nctx2[hh])
                done_norms[nqt] += 1
                if done_norms[nqt] == 2 and nqt not in d_emitted:
                    d_emitted.add(nqt)
                    for nb in range(4 * nqt, 4 * nqt + 4):
                        filler_q.append(
                            lambda nb=nb: emit_outproj_nb(nb, tail)
                        )

            prev_pv = None  # (pair, qt, nchunks, pv_descs, ctx2)
            norm_q = []  # normalize one unit behind the PV
            for ui, (pair, qt, nchunks) in enumerate(units):
                if ui == 1:
                    # bulk xt/wout DMAs, gated on the first K cast so they
                    # don't steal HBM bandwidth from the startup prefix. The
                    # gate binds via a WAW hazard: a tiny copy that reads
                    # kt_pair (produced by the K cast) and writes into the
                    # DMA's destination region, so the scheduler cannot
                    # hoist the DMA ahead of it.
                    for e in range(8):
                        nc.gpsimd.tensor_copy(
                            xt[e][:, 512:516], kt_pair[0][:, 0:4]
                        )
                        nc.gpsimd.dma_start(
                            xt[e][:, 512:N], xt_d[e * 128:(e + 1) * 128, 512:N]
                        )
                    nc.gpsimd.tensor_copy(wo_t[:, 0, 0:4], kt_pair[0][:, 0:4])
                    nc.gpsimd.dma_start(
                        wo_t[:].rearrange("p c m -> p (c m)"), wout_d[:]
                    )
                pv = emit_st_exp(ui, pair, qt, nchunks, prev_pv)
                # normalize runs ONE unit behind its PV (which was just
                # riffled into this unit): popping here keeps the single
                # ctx PSUM buffer per head from serializing the next PV
                # against a 2-generation-old reader, and unlocks outproj
                # filler a unit earlier.
                if prev_pv is not None:
                    ppair, pqt, pn, ppv, pctx2 = prev_pv
                    norm_q.append((ppair, pqt, pctx2))
                if norm_q:
                    pop_norm()
                ctx2 = [
                    ps_ctx.tile([128, 512], F32, tag=f"ctx{hh}", name=f"ctx{hh}")
                    for hh in range(2)
                ]
                prev_pv = (pair, qt, nchunks, pv, ctx2)
            # flush leftover filler (outproj blocks of earlier q-tiles)
            # BEFORE the final PV/normalize chain so the PE stays busy —
            # an idle sliver here re-throttles the clock for the whole tail
            while filler_q:
                filler_q.pop(0)()
            ppair, pqt, pn, ppv, pctx2 = prev_pv
            emit_pv(ppair, pqt, pn, ppv, pctx2)
            norm_q.append((ppair, pqt, pctx2))
            while norm_q:
                pop_norm(tail=True)
            while filler_q:
                filler_q.pop(0)()

            pt_cm.__exit__(None, None, None)
            xt_cm.__exit__(None, None, None)

    nc.compile()
    return nc


_PROGRAM_CACHE = {}


def kernel(x, attention_mask, W_Q, W_K, W_V, W_out, b_out):
    global LAST_RESULTS
    from concourse.bass_utils import run_bass_kernel_spmd

    x = np.ascontiguousarray(x, dtype=np.float32)
    attention_mask = np.asarray(attention_mask)
    lengths = attention_mask.astype(np.int64).sum(axis=1)
    kb_max = int(math.ceil(lengths.max() / KBLK))
    jpad_min = int(lengths.min() // KBLK)

    key = (kb_max, jpad_min)
    if key not in _PROGRAM_CACHE:
        _PROGRAM_CACHE[key] = _build_program(kb_max, jpad_min)
    nc = _PROGRAM_CACHE[key]

    # host-side input prep (matmul operands pre-cast to bf16)
    import ml_dtypes
    BF = ml_dtypes.bfloat16
    xT = [np.ascontiguousarray(x[b].T.astype(BF)) for b in range(B)]
    wqT = np.ascontiguousarray(np.asarray(W_Q, dtype=np.float32).T.astype(BF))
    wkT = np.ascontiguousarray(np.asarray(W_K, dtype=np.float32).T.astype(BF))
    wvT = np.ascontiguousarray(np.asarray(W_V, dtype=np.float32).T.astype(BF))
    woT = np.ascontiguousarray(np.asarray(W_out, dtype=np.float32).T.astype(BF))
    # padbias[p, j] = 0 if key j*128+p is real else -30000
    padb = [
        np.ascontiguousarray(
            np.where(attention_mask[b].reshape(16, 128).T != 0, 0.0, NEGB)
        ).astype(np.float32)
        for b in range(B)
    ]
    # trimask[p, u] = 0 if u < p + 384 else 1; slice [384-128d : 896-128d]
    # gives the causal 0/1 mask for a diagonal block with offset 128d,
    # duplicated along the 2-head axis -> [128, 2*896].
    pp = np.arange(128)[:, None]
    uu = np.arange(896)[None, :]
    trimask1 = np.where(uu < pp + 384, 0.0, 1.0)
    trimask = np.ascontiguousarray(
        np.broadcast_to(trimask1[:, None, :], (128, 2, 896)).reshape(128, 1792)
    ).astype(BF)
    ones65 = np.ones((128, 64), dtype=BF)

    def pack_w(w):  # [1024, 256] -> on-chip [128, 8, 256] layout, flattened
        return np.ascontiguousarray(
            w.reshape(8, 128, 256).transpose(1, 0, 2).reshape(128, 2048)
        )

    def pack_wo(w):  # [256, 1024] -> on-chip [128, 2, 1024] layout, flattened
        return np.ascontiguousarray(
            w.reshape(2, 128, 1024).transpose(1, 0, 2).reshape(128, 2048)
        )

    in_maps = []
    for c in range(NCORES):
        b, g = divmod(c, 4)
        sl = slice(g * 256, (g + 1) * 256)
        in_maps.append(
            {
                "xt": xT[b],
                "wq": pack_w(wqT[:, sl]),
                "wk": pack_w(wkT[:, sl]),
                "wv": pack_w(wvT[:, sl]),
                "wout": pack_wo(woT[sl, :]),
                "padbias": padb[b],
                "trimask": trimask,
                "ones65": ones65,
            }
        )

    trace = bool(int(os.environ.get("KERNEL_TRACE", "0")))
    ncores_run = int(os.environ.get("KERNEL_NCORES", str(NCORES)))
    res = run_bass_kernel_spmd(
        nc,
        in_maps[:ncores_run],
        core_ids=list(range(ncores_run)),
        trace=trace,
        trace_cores=list(range(ncores_run)) if trace else None,
    )
    LAST_RESULTS = res

    out = np.zeros((B, N, D), dtype=np.float32)
    for c in range(len(res.results)):
        out[c // 4] += res.results[c]["out"].astype(np.float32)
    out += np.asarray(b_out, dtype=np.float32)[None, None, :]
    return out



# revision 88
# speedup vs baseline: 1.1103x; 1.1103x over previous
"""Trainium2 Bass kernel for causal+padded multi-head attention.

Problem: B=2, N=2048, D=1024, H=16 heads (DK=64), fp32 I/O.
  out = softmax(mask(x Wq^T (x Wk^T)^T) / sqrt(DK)) (x Wv^T) Wout^T + b_out

Sharding (8 cores): core c handles batch b=c//4 and heads [4*(c%4), 4*(c%4)+4).
Each core computes a partial output [N, D] (its 4 heads' contribution through
the output projection); the host sums the 4 partials per batch and adds b_out.

On-device layout (per core):
  xT   [1024, 2048]  (host-pretransposed x[b])
  QT/KT stored transposed [dk, n] as head-pair tiles [128, 2048]
  V    stored natural as [128(keys), 16 blocks, 4 heads, 65] with a ones
       column appended (col 64) so P@V' also yields the softmax denominator.
  S^T  computed per (head-pair, q-tile 512, key-block 128) as [128, 2, 512]
       in PSUM: matmul(lhsT=KT slice [64,128], rhs=QT slice [64,512]); the
       two heads sit at base partitions 0/64 so their matmuls row-tile and
       run concurrently on the PE.
       Causal masking = additive -30000 on PSUM (DVE); padding mask is a
       per-key bias fused into the exp; one exp(0.125*s + bias) on ScalarE
       writes P^T straight to SBUF as bf16.
  ctx'^T [65, 512] accumulated in PSUM over key blocks:
       matmul(lhsT=V' [128,65], rhs=P^T [128,512]); PV matmuls are emitted
       one unit behind their exps so the in-order PE never waits on ScalarE
       (idle slivers would re-throttle the HAM clock gate to 1.2 GHz).
  Normalization: r = recip(rowsum) on the [1,512] denominator row (DVE),
       partition-broadcast to [64,512] (GpSimd), one DVE multiply;
       software-pipelined one unit behind the PV.
  Out projection: matmul(lhsT=ctxT [128,128], rhs=WoutT [128,512]) acc over
       the two head-pair chunks.

All matmul operands are bf16 (pre-rounded on host for the inputs; on-device
casts for intermediates); accumulation is fp32 in PSUM, and the softmax /
masking / normalization arithmetic is fp32. float32r was measured on this
hardware at 2 cycles/row with a serial (non-FWL) weight load that starves
the HAM activity monitor - bf16 is ~2.7x faster per matmul in practice.
"""

import math
import os

import numpy as np

B, N, D, H = 2, 2048, 1024, 16
DK = D // H  # 64
NCORES = 8
HEADS_PER_CORE = 4
QTILE = 512
KBLK = 128
NEG = -30000.0
NEGB = -3750.0  # pad bias applied after the 0.125 scale inside exp
SCALE = 1.0 / math.sqrt(float(DK))  # 0.125

# Set by run() when tracing is enabled (test.py reads this).
LAST_RESULTS = None


def _build_program(kb_max: int, jpad_min: int):
    import concourse.tile as tile
    from concourse import bacc, mybir

    F32 = mybir.dt.float32
    F32R = mybir.dt.float32r
    BF16 = mybir.dt.bfloat16
    EXP = mybir.ActivationFunctionType.Exp
    ADD = mybir.AluOpType.add

    nc = bacc.Bacc(None)

    # weights arrive pre-arranged on the host into the on-chip layout so
    # every DMA is a contiguous [128, x] transfer (strided gathers stall the
    # critical first-projection path)
    xt_d = nc.dram_tensor("xt", [D, N], BF16, kind="ExternalInput")
    wq_d = nc.dram_tensor("wq", [128, 2048], BF16, kind="ExternalInput")
    wk_d = nc.dram_tensor("wk", [128, 2048], BF16, kind="ExternalInput")
    wv_d = nc.dram_tensor("wv", [128, 2048], BF16, kind="ExternalInput")
    wout_d = nc.dram_tensor("wout", [128, 2048], BF16, kind="ExternalInput")
    padb_d = nc.dram_tensor("padbias", [128, 16], F32, kind="ExternalInput")
    trimask_d = nc.dram_tensor("trimask", [128, 1792], BF16, kind="ExternalInput")
    ones_d = nc.dram_tensor("ones65", [128, 64], BF16, kind="ExternalInput")
    out_d = nc.dram_tensor("out", [N, D], BF16, kind="ExternalOutput")

    NB = N // KBLK  # 16 key/row blocks
    NQT = N // QTILE  # 4 q tiles

    with tile.TileContext(nc) as tc:
        with (
            tc.tile_pool(name="w", bufs=1) as w_pool,
            tc.tile_pool(name="big", bufs=1) as big_pool,
            tc.tile_pool(name="work", bufs=3) as work_pool,
            tc.tile_pool(name="ps_main", bufs=3, space="PSUM") as ps_main,
            tc.tile_pool(name="ps_ctx", bufs=1, space="PSUM") as ps_ctx,
        ):
            # ---- load inputs ----
            # Order matters: wq/wk then the xt chunks gate the first Q/K
            # projection, which gates the whole pipeline. Everything else
            # streams behind them.
            xt_cm = tc.tile_pool(name="xt", bufs=8)
            xt_pool = xt_cm.__enter__()
            wq_t = w_pool.tile([128, 8, 256], BF16, tag="wq")
            wk_t = w_pool.tile([128, 8, 256], BF16, tag="wk")
            wv_t = w_pool.tile([128, 8, 256], BF16, tag="wv")
            wo_t = w_pool.tile([128, 2, D], BF16, tag="wo")
            padb_t = w_pool.tile([128, 16], F32, tag="padb")
            trimask_t = w_pool.tile([128, 2, 896], BF16, tag="trimask")
            # The first q-tile of x streams per-chunk so the Q projection's
            # accumulation steps pipeline with DMA arrival. The critical
            # startup prefix (wq/wk/xt q-tile 0) is split across the sync
            # and gpsimd DMA queues — a single queue tops out well below
            # HBM bandwidth. Everything the startup doesn't need is gated
            # behind the first K cast.
            xt = [
                xt_pool.tile([128, N], BF16, tag="xt", name=f"xt{e}")
                for e in range(8)
            ]
            # interleave the weight-chunk and x-chunk transfers so the first
            # Q/K accumulation steps start on the first ~0.3MB instead of
            # waiting for whole tensors
            # prefix split across three DMA queues (sync/vector/gpsimd) —
            # one queue tops out at ~160GB/s, a third of per-core HBM
            for e in range(8):
                nc.sync.dma_start(wq_t[:, e, :], wq_d[:, e * 256:(e + 1) * 256])
                eng = nc.sync if e % 2 == 0 else nc.scalar
                eng.dma_start(
                    xt[e][:, 0:512], xt_d[e * 128:(e + 1) * 128, 0:512]
                )
                nc.gpsimd.dma_start(wk_t[:, e, :], wk_d[:, e * 256:(e + 1) * 256])
            nc.sync.dma_start(wv_t[:].rearrange("p e m -> p (e m)"), wv_d[:])
            nc.sync.dma_start(padb_t[:], padb_d[:])
            nc.gpsimd.dma_start(
                trimask_t[:], trimask_d[:].rearrange("p (h u) -> p h u", h=2)
            )

            # Warm the ScalarE activation table (exp set) during the input
            # DMA window so the first real exp doesn't eat the ~2.7us load.
            warm_in = w_pool.tile([1, 8], F32, tag="warm_in")
            warm_out = w_pool.tile([1, 8], F32, tag="warm_out")
            nc.gpsimd.memset(warm_in[:], 0.0)
            nc.scalar.activation(warm_out[:], warm_in[:], EXP)

            # V' tile: [keys 128, key-block 16, head 4, 128]; col 64 <- ones,
            # cols 65-127 <- 0 (padding to 128 weight columns keeps FWL on for
            # the PV matmuls; PSUM rows 65-127 of ctx' are dead).
            v4 = big_pool.tile([128, NB, 4, 128], BF16, tag="v4")
            nc.vector.memset(v4[:, :, :, 65:128], 0.0)
            nc.sync.dma_start(
                v4[:, :, :, 64:65],
                ones_d[:].rearrange("p (b h o) -> p b h o", h=4, o=1),
            )

            qt_pair = [big_pool.tile([128, N], BF16, tag=f"qt{p}", name=f"qt{p}") for p in range(2)]
            kt_pair = [big_pool.tile([128, N], BF16, tag=f"kt{p}", name=f"kt{p}") for p in range(2)]
            ctx_pair = [big_pool.tile([128, N], BF16, tag=f"ctx{p}", name=f"ctx{p}") for p in range(2)]

            pt_cm = tc.tile_pool(name="pt", bufs=26)
            pt_pool = pt_cm.__enter__()

            # ---- projection tasks, riffled into the attention stream ----
            # QT/KT: [dk(128 = 2 heads), n] = (W.T chunk)^T @ xT
            def xt_slice(e, c0, c1):
                return xt[e][:, c0:c1]

            def emit_qk_proj(w_t, dst, pair, nq):
                ps = ps_main.tile([128, 2, 512], F32, tag="blk", name="blk")[:, 0, :]
                for e in range(8):
                    nc.tensor.matmul(
                        ps[:],
                        w_t[:, e, pair * 128:(pair + 1) * 128],
                        xt_slice(e, nq * 512, (nq + 1) * 512),
                        start=(e == 0),
                        stop=(e == 7),
                    )
                nc.vector.tensor_copy(dst[pair][:, nq * 512:(nq + 1) * 512], ps[:])

            def emit_qk_fused(pair, nq):
                # prologue: Q and K accumulate per x-chunk in lockstep so
                # both finish ~one matmul after the last x chunk arrives
                psq = ps_main.tile([128, 2, 512], F32, tag="blk", name="psq")[:, 0, :]
                psk = ps_main.tile([128, 2, 512], F32, tag="blk", name="psk")[:, 0, :]
                for e in range(8):
                    nc.tensor.matmul(
                        psq[:],
                        wq_t[:, e, pair * 128:(pair + 1) * 128],
                        xt_slice(e, nq * 512, (nq + 1) * 512),
                        start=(e == 0),
                        stop=(e == 7),
                    )
                    nc.tensor.matmul(
                        psk[:],
                        wk_t[:, e, pair * 128:(pair + 1) * 128],
                        xt_slice(e, nq * 512, (nq + 1) * 512),
                        start=(e == 0),
                        stop=(e == 7),
                    )
                nc.vector.tensor_copy(
                    kt_pair[pair][:, nq * 512:(nq + 1) * 512], psk[:]
                )
                nc.vector.tensor_copy(
                    qt_pair[pair][:, nq * 512:(nq + 1) * 512], psq[:]
                )

            # V natural: [n-block, 4*64] = xT-chunk^T @ WvT-chunk
            def emit_v_proj(nb):
                ps = ps_main.tile([128, 2, 512], F32, tag="blk", name="blk")[:, 0, 0:256]
                for e in range(8):
                    nc.tensor.matmul(
                        ps[:],
                        xt_slice(e, nb * 128, (nb + 1) * 128),
                        wv_t[:, e, :],
                        start=(e == 0),
                        stop=(e == 7),
                    )
                nc.vector.tensor_copy(
                    v4[:, nb, :, 0:64],
                    ps[:].rearrange("p (h d) -> p h d", h=4),
                )

            # Unit order: q0, q2, q3, q1. Starting on q0 keeps the cold-clock
            # prologue to two projection groups; the exp-heavy q2/q3 stretch
            # sits mid-stream where V-block/outproj filler is plentiful; the
            # small-PV q1 tail keeps the post-last-exp drain short.
            # Each projection task carries a (unit, slot) deadline: its S^T
            # needs Q at slot 0 and K tile t by slot 4t; the riffled PV of
            # unit u-1 needs V block nb by slot nb of unit u. Tasks drain
            # lazily at ~1 per 2 chunk slots so filler PE work carries
            # forward into the exp-heavy units; deadlines force correctness.
            qt_order = [0, 2, 3, 1]
            units = [
                (pair, qt, min(4 * qt + 4, kb_max))
                for qt in qt_order
                for pair in range(2)
            ]

            def t_q(pair, r):
                return lambda: emit_qk_proj(wq_t, qt_pair, pair, r)

            def t_k(pair, r):
                return lambda: emit_qk_proj(wk_t, kt_pair, pair, r)

            def t_v(nb):
                return lambda: emit_v_proj(nb)

            tasks = []  # (unit, slot, thunk)
            k_done = [set(), set()]
            v_done = set()
            for ui, (pair, qt, nchunks) in enumerate(units):
                if ui == 0:
                    # fused Q+K prologue for the very first unit
                    k_done[pair].add(qt)
                    tasks.append((0, 0, lambda p=pair, r=qt: emit_qk_fused(p, r)))
                else:
                    tasks.append((ui, 0, t_q(pair, qt)))
                for t in range(qt + 1):
                    if t not in k_done[pair]:
                        k_done[pair].add(t)
                        tasks.append((ui, 4 * t, t_k(pair, t)))
                if ui + 1 < len(units):
                    # V blocks consumed by this unit's PV, riffled in unit+1
                    for nb in range(nchunks):
                        if nb not in v_done:
                            v_done.add(nb)
                            tasks.append((ui + 1, nb, t_v(nb)))
            tasks.sort(key=lambda e: (e[0], e[1]))
            proj_fifo = list(tasks)
            filler_q = []  # deadline-free PE filler (outproj nb-blocks)

            def drain_due(ui, j):
                while proj_fifo and (proj_fifo[0][0], proj_fifo[0][1]) <= (ui, j):
                    proj_fifo.pop(0)[2]()

            def drain_lazy():
                if proj_fifo:
                    proj_fifo.pop(0)[2]()
                elif filler_q:
                    filler_q.pop(0)()

            # ---- attention, head pairs interleaved ----
            # A unit is (head-pair, q-tile). The two heads' S^T matmuls sit
            # at base partitions 0 / 64 (row groups 0-63 / 64-127), so they
            # execute concurrently on the PE and their weight loads overlap
            # the other head's matmul — no LDW bubble, HAM stays warm.
            # PV matmuls run one unit behind their exps so the in-order PE
            # never drains waiting on ScalarE.
            def emit_normalize(pair, hh, qt, ctx_ps):
                # the craw copy doubles as the PSUM release: it is the only
                # reader of ctx_ps rows 0-63, so the next unit's PV (same
                # single-buffered bank) can start as soon as it completes
                # instead of waiting for the whole normalize chain
                hp = slice(64 * hh, 64 * hh + 64)
                craw = work_pool.tile([64, 512], F32, tag="craw", name="craw")
                nc.vector.tensor_copy(craw[:], ctx_ps[0:64, :])
                rden = work_pool.tile([1, 512], F32, tag="rden", name="rden")
                nc.scalar.copy(rden[:], ctx_ps[64:65, :])
                rrec = work_pool.tile([1, 512], F32, tag="rrec", name="rrec")
                nc.vector.reciprocal_approx_fast(rrec[:], rden[:])
                rbr = work_pool.tile([64, 512], F32, tag="rbr", name="rbr")
                nc.gpsimd.partition_broadcast(rbr[:], rrec[:])
                nc.vector.tensor_mul(
                    ctx_pair[pair][hp, qt * 512:(qt + 1) * 512],
                    craw[0:64, :],
                    rbr[:],
                )

            def emit_st_exp(ui, pair, qt, nchunks, prev, self_ctx2=None):
                """S^T + mask + exp for both heads, with the previous unit's
                PV matmuls riffled in (they are long-ready and fill the PE
                slots where S^T would stall on the exp pipeline). Returns
                PV descriptors."""
                if prev is None:
                    ppv = []
                else:
                    ppair, pqt, pn, ppv, pctx2 = prev

                def rif(k):
                    # emit previous-unit PV chunks up to index k
                    while ppv and ppv[0][0] <= k:
                        jj, ptt, poff = ppv.pop(0)
                        for hh in range(2):
                            nc.tensor.matmul(
                                pctx2[hh][:, poff:],
                                v4[:, jj, 2 * ppair + hh, :],
                                ptt[:, hh, poff:],
                                start=(jj == 0),
                                stop=(jj == pn - 1),
                                skip_group_check=True,
                            )

                pv = []
                # drain filler every slot when the riffled previous unit is
                # smaller than this one (less PV fuel per slot) and in the
                # last units (so no PE work trails the final exps);
                # otherwise every other slot
                hungry = (
                    ui >= len(units) - 2
                    or prev is None
                    or prev[2] < nchunks
                )
                for j in range(nchunks):
                    drain_due(ui, j)
                    rif(j)
                    if hungry or j % 2 == 1:
                        drain_lazy()
                    d = j - 4 * qt
                    # exact-causal column trim (keep matmul N >= 256)
                    off = 128 * d if d >= 1 else 0
                    st_ps = ps_main.tile([128, 2, 512], F32, tag="blk", name="blk")
                    for hh in range(2):
                        hp = slice(64 * hh, 64 * hh + 64)
                        nc.tensor.matmul(
                            st_ps[:, hh, off:],
                            kt_pair[pair][hp, j * 128:(j + 1) * 128],
                            qt_pair[pair][hp, qt * 512 + off:(qt + 1) * 512],
                            start=True,
                            stop=True,
                        )
                    pt_t = pt_pool.tile([128, 2, 512], BF16, tag="pt")
                    kw = {}
                    if j >= jpad_min:  # per-key pad bias (same for both heads)
                        kw["bias"] = padb_t[:, j:j + 1]
                    nc.scalar.activation(
                        pt_t[:, :, off:], st_ps[:, :, off:], EXP, scale=SCALE, **kw
                    )
                    if d >= 0:
                        # causal mask as a post-exp 0/1 multiply; per-head 2D
                        # slices keep the DVE in its fast dense mode (a 3D
                        # strided AP drops it to 1x). With off = 128*d the
                        # masked triangle lies entirely in cols [off, off+128)
                        u0 = 384 - 128 * d + off
                        w = min(128, 512 - off)
                        for hh in range(2):
                            nc.vector.tensor_mul(
                                pt_t[:, hh, off:off + w],
                                pt_t[:, hh, off:off + w],
                                trimask_t[:, hh, u0:u0 + w],
                            )
                    pv.append((j, pt_t, off))
                    if self_ctx2 is not None and len(pv) >= 2:
                        # final unit: riffle its own PV one chunk behind the
                        # exps so the tail isn't a serial PV stream after
                        # the last exp
                        jj, ptt, poff = pv.pop(0)
                        for hh in range(2):
                            nc.tensor.matmul(
                                self_ctx2[hh][:, poff:],
                                v4[:, jj, 2 * pair + hh, :],
                                ptt[:, hh, poff:],
                                start=(jj == 0),
                                stop=(jj == nchunks - 1),
                                skip_group_check=True,
                            )
                rif(10 ** 9)
                return pv

            def emit_pv(pair, qt, nchunks, pv, ctx2):
                for j, pt_t, off in pv:
                    for hh in range(2):
                        nc.tensor.matmul(
                            ctx2[hh][:, off:],
                            v4[:, j, 2 * pair + hh, :],
                            pt_t[:, hh, off:],
                            start=(j == 0),
                            stop=(j == nchunks - 1),
                            skip_group_check=True,
                        )

            done_norms = {q: 0 for q in range(NQT)}
            d_emitted = set()

            def emit_outproj_nb(nb, split):
                # output projection for one 128-token block; `split` sends
                # the fc=1 PSUM->SBUF cast to ScalarE (tail drain, when
                # ScalarE has gone idle) instead of DVE.
                osb = work_pool.tile([128, D], BF16, tag="osb", name="osb")
                for fc in range(2):
                    ps = ps_main.tile(
                        [128, 2, 512], F32, tag="blk", name="blk"
                    )[:, 0, :]
                    for pr2 in range(2):
                        nc.tensor.matmul(
                            ps[:],
                            ctx_pair[pr2][:, nb * 128:(nb + 1) * 128],
                            wo_t[:, pr2, fc * 512:(fc + 1) * 512],
                            start=(pr2 == 0),
                            stop=(pr2 == 1),
                        )
                    if split and fc == 1:
                        nc.scalar.copy(osb[:, fc * 512:(fc + 1) * 512], ps[:])
                    else:
                        nc.vector.tensor_copy(
                            osb[:, fc * 512:(fc + 1) * 512], ps[:]
                        )
                if split:
                    # tail: split the write across two DMA queues (one queue
                    # tops out at ~140GB/s; gpsimd's queue is free by now)
                    nc.sync.dma_start(
                        out_d[nb * 128:(nb + 1) * 128, 0:512], osb[:, 0:512]
                    )
                    nc.gpsimd.dma_start(
                        out_d[nb * 128:(nb + 1) * 128, 512:D], osb[:, 512:D]
                    )
                else:
                    nc.sync.dma_start(out_d[nb * 128:(nb + 1) * 128, :], osb[:])

            def pop_norm(tail=False):
                npair, nqt, nctx2 = norm_q.pop(0)
                for hh in range(2):
                    emit_normalize(npair, hh, nqt, nctx2[hh])
                done_norms[nqt] += 1
                if done_norms[nqt] == 2 and nqt not in d_emitted:
                    d_emitted.add(nqt)
                    for nb in range(4 * nqt, 4 * nqt + 4):
                        filler_q.append(
                            lambda nb=nb: emit_outproj_nb(nb, tail)
                        )

            prev_pv = None  # (pair, qt, nchunks, pv_descs, ctx2)
            norm_q = []  # normalize one unit behind the PV
            for ui, (pair, qt, nchunks) in enumerate(units):
                if ui == 1:
                    # bulk xt/wout DMAs, gated on the first K cast so they
                    # don't steal HBM bandwidth from the startup prefix. The
                    # gate binds via a WAW hazard: a tiny copy that reads
                    # kt_pair (produced by the K cast) and writes into the
                    # DMA's destination region, so the scheduler cannot
                    # hoist the DMA ahead of it.
                    for e in range(8):
                        nc.gpsimd.tensor_copy(
                            xt[e][:, 512:516], kt_pair[0][:, 0:4]
                        )
                        nc.gpsimd.dma_start(
                            xt[e][:, 512:N], xt_d[e * 128:(e + 1) * 128, 512:N]
                        )
                    nc.gpsimd.tensor_copy(wo_t[:, 0, 0:4], kt_pair[0][:, 0:4])
                    nc.gpsimd.dma_start(
                        wo_t[:].rearrange("p c m -> p (c m)"), wout_d[:]
                    )
                pv = emit_st_exp(ui, pair, qt, nchunks, prev_pv)
                # normalize runs ONE unit behind its PV (which was just
                # riffled into this unit): popping here keeps the single
                # ctx PSUM buffer per head from serializing the next PV
                # against a 2-generation-old reader, and unlocks outproj
                # filler a unit earlier.
                if prev_pv is not None:
                    ppair, pqt, pn, ppv, pctx2 = prev_pv
                    norm_q.append((ppair, pqt, pctx2))
                if norm_q:
                    pop_norm()
                ctx2 = [
                    ps_ctx.tile([128, 512], F32, tag=f"ctx{hh}", name=f"ctx{hh}")
                    for hh in range(2)
                ]
                prev_pv = (pair, qt, nchunks, pv, ctx2)
            # flush leftover filler (outproj blocks of earlier q-tiles)
            # BEFORE the final PV/normalize chain so the PE stays busy —
            # an idle sliver here re-throttles the clock for the whole tail
            while filler_q:
                filler_q.pop(0)()
            ppair, pqt, pn, ppv, pctx2 = prev_pv
            emit_pv(ppair, pqt, pn, ppv, pctx2)
            norm_q.append((ppair, pqt, pctx2))
            while norm_q:
                pop_norm(tail=True)
            while filler_q:
                filler_q.pop(0)()

            pt_cm.__exit__(None, None, None)
            xt_cm.__exit__(None, None, None)

    nc.compile()
    return nc


_PROGRAM_CACHE = {}


def kernel(x, attention_mask, W_Q, W_K, W_V, W_out, b_out):
    global LAST_RESULTS
    from concourse.bass_utils import run_bass_kernel_spmd

    x = np.ascontiguousarray(x, dtype=np.float32)
    attention_mask = np.asarray(attention_mask)
    lengths = attention_mask.astype(np.int64).sum(axis=1)
    kb_max = int(math.ceil(lengths.max() / KBLK))
    jpad_min = int(lengths.min() // KBLK)

    key = (kb_max, jpad_min)
    if key not in _PROGRAM_CACHE:
        _PROGRAM_CACHE[key] = _build_program(kb_max, jpad_min)
    nc = _PROGRAM_CACHE[key]

    # host-side input prep (matmul operands pre-cast to bf16)
    import ml_dtypes
    BF = ml_dtypes.bfloat16
    xT = [np.ascontiguousarray(x[b].T.astype(BF)) for b in range(B)]
    wqT = np.ascontiguousarray(np.asarray(W_Q, dtype=np.float32).T.astype(BF))
    wkT = np.ascontiguousarray(np.asarray(W_K, dtype=np.float32).T.astype(BF))
    wvT = np.ascontiguousarray(np.asarray(W_V, dtype=np.float32).T.astype(BF))
    woT = np.ascontiguousarray(np.asarray(W_out, dtype=np.float32).T.astype(BF))
    # padbias[p, j] = 0 if key j*128+p is real else -30000
    padb = [
        np.ascontiguousarray(
            np.where(attention_mask[b].reshape(16, 128).T != 0, 0.0, NEGB)
        ).astype(np.float32)
        for b in range(B)
    ]
    # trimask[p, u] = 0 if u < p + 384 else 1; slice [384-128d : 896-128d]
    # gives the causal 0/1 mask for a diagonal block with offset 128d,
    # duplicated along the 2-head axis -> [128, 2*896].
    pp = np.arange(128)[:, None]
    uu = np.arange(896)[None, :]
    trimask1 = np.where(uu < pp + 384, 0.0, 1.0)
    trimask = np.ascontiguousarray(
        np.broadcast_to(trimask1[:, None, :], (128, 2, 896)).reshape(128, 1792)
    ).astype(BF)
    ones65 = np.ones((128, 64), dtype=BF)

    def pack_w(w):  # [1024, 256] -> on-chip [128, 8, 256] layout, flattened
        return np.ascontiguousarray(
            w.reshape(8, 128, 256).transpose(1, 0, 2).reshape(128, 2048)
        )

    def pack_wo(w):  # [256, 1024] -> on-chip [128, 2, 1024] layout, flattened
        return np.ascontiguousarray(
            w.reshape(2, 128, 1024).transpose(1, 0, 2).reshape(128, 2048)
        )

    in_maps = []
    for c in range(NCORES):
        b, g = divmod(c, 4)
        sl = slice(g * 256, (g + 1) * 256)
        in_maps.append(
            {
                "xt": xT[b],
                "wq": pack_w(wqT[:, sl]),
                "wk": pack_w(wkT[:, sl]),
                "wv": pack_w(wvT[:, sl]),
                "wout": pack_wo(woT[sl, :]),
                "padbias": padb[b],
                "trimask": trimask,
                "ones65": ones65,
            }
        )

    trace = bool(int(os.environ.get("KERNEL_TRACE", "0")))
    ncores_run = int(os.environ.get("KERNEL_NCORES", str(NCORES)))
    res = run_bass_kernel_spmd(
        nc,
        in_maps[:ncores_run],
        core_ids=list(range(ncores_run)),
        trace=trace,
        trace_cores=list(range(ncores_run)) if trace else None,
    )
    LAST_RESULTS = res

    out = np.zeros((B, N, D), dtype=np.float32)
    for c in range(len(res.results)):
        out[c // 4] += res.results[c]["out"].astype(np.float32)
    out += np.asarray(b_out, dtype=np.float32)[None, None, :]
    return out



# revision 90
# speedup vs baseline: 1.1199x; 1.0086x over previous
"""Trainium2 Bass kernel for causal+padded multi-head attention.

Problem: B=2, N=2048, D=1024, H=16 heads (DK=64), fp32 I/O.
  out = softmax(mask(x Wq^T (x Wk^T)^T) / sqrt(DK)) (x Wv^T) Wout^T + b_out

Sharding (8 cores): core c handles batch b=c//4 and heads [4*(c%4), 4*(c%4)+4).
Each core computes a partial output [N, D] (its 4 heads' contribution through
the output projection) in bf16; the host sums the 4 partials per batch and
adds b_out.

Design (per core) — a single software-pipelined stream; the projections are
NOT a separate phase but are riffled into the attention schedule as PE
filler so ScalarE (exp) ramps up ~15us in and the PE never idles long
enough to re-throttle the HAM clock gate to 1.2 GHz:

  xT   [1024, 2048] (host-pretransposed x[b]); QT/KT transposed [dk, n] as
       head-pair tiles [128, 2048]; V natural as [128(keys), 16, 4, 128]
       where col 64 is a ones column (so P@V' also yields the softmax
       denominator) and cols 65-127 are zero padding (keeps NumWeights==128
       so FWL hides the PV weight loads).
  Units are (head-pair, q-tile 512), processed in q-tile order 0,2,3,1:
       q0 first keeps the cold-clock prologue to one fused Q+K group; the
       exp-heavy q2/q3 stretch sits mid-stream where V-block/outproj filler
       is plentiful; the small-PV q1 tail keeps the post-last-exp drain
       short.
  S^T  per (unit, key-block 128) as [128, 2, 512] PSUM: matmul(lhsT=KT
       slice [64,128], rhs=QT slice [64,512]); the two heads sit at base
       partitions 0/64 so their matmuls row-tile and run concurrently.
       One exp(0.125*s + padbias) on ScalarE (bias = per-key pad mask)
       writes P^T to SBUF bf16; causal masking is a post-exp 0/1 multiply
       on DVE (per-head 2D slices keep the fast dense mode).
  PV   [128, 512] PSUM accumulated over key blocks, riffled one unit behind
       the exps between the next unit's S^T matmuls.
  Norm (one unit behind PV): DVE copy of ctx rows (doubles as the PSUM
       release), ScalarE copy of the denominator row, reciprocal_approx_fast
       (DVE; input must be a base-partition-0 AP), GpSimd partition
       broadcast, one DVE multiply -> ctx bf16.
  Outproj per 128-token block, queued as deadline-free PE filler and
       drained into exp-heavy units; PSUM->SBUF casts on DVE (tail: split
       DVE/ScalarE), output written bf16, split across two DMA queues in
       the tail (one queue tops out at ~140GB/s).
  DMA  the critical prefix (wq/wk interleaved with xt q-tile 0) is split
       across the sync/scalar/gpsimd queues; all weights arrive host-packed
       in the on-chip layout so every transfer is contiguous; the bulk of
       xt and wout are gated behind the first K cast (WAW hazard via a tiny
       copy) so they don't steal HBM bandwidth from the prefix; the ScalarE
       exp table is prefetched with a dummy activation at t=0.

All matmul operands are bf16 (pre-rounded on host for the inputs; on-device
casts for intermediates); accumulation is fp32 in PSUM, and the softmax /
normalization arithmetic is fp32.
"""

import math
import os

import numpy as np

B, N, D, H = 2, 2048, 1024, 16
DK = D // H  # 64
NCORES = 8
HEADS_PER_CORE = 4
QTILE = 512
KBLK = 128
NEG = -30000.0
NEGB = -3750.0  # pad bias applied after the 0.125 scale inside exp
SCALE = 1.0 / math.sqrt(float(DK))  # 0.125

# Set by run() when tracing is enabled (test.py reads this).
LAST_RESULTS = None


def _build_program(kb_max: int, jpad_min: int):
    import concourse.tile as tile
    from concourse import bacc, mybir

    F32 = mybir.dt.float32
    F32R = mybir.dt.float32r
    BF16 = mybir.dt.bfloat16
    EXP = mybir.ActivationFunctionType.Exp
    ADD = mybir.AluOpType.add

    nc = bacc.Bacc(None)

    # weights arrive pre-arranged on the host into the on-chip layout so
    # every DMA is a contiguous [128, x] transfer (strided gathers stall the
    # critical first-projection path)
    xt_d = nc.dram_tensor("xt", [D, N], BF16, kind="ExternalInput")
    wq_d = nc.dram_tensor("wq", [128, 2048], BF16, kind="ExternalInput")
    wk_d = nc.dram_tensor("wk", [128, 2048], BF16, kind="ExternalInput")
    wv_d = nc.dram_tensor("wv", [128, 2048], BF16, kind="ExternalInput")
    wout_d = nc.dram_tensor("wout", [128, 2048], BF16, kind="ExternalInput")
    padb_d = nc.dram_tensor("padbias", [128, 16], F32, kind="ExternalInput")
    trimask_d = nc.dram_tensor("trimask", [128, 1792], BF16, kind="ExternalInput")
    ones_d = nc.dram_tensor("ones65", [128, 64], BF16, kind="ExternalInput")
    out_d = nc.dram_tensor("out", [N, D], BF16, kind="ExternalOutput")

    NB = N // KBLK  # 16 key/row blocks
    NQT = N // QTILE  # 4 q tiles

    with tile.TileContext(nc) as tc:
        with (
            tc.tile_pool(name="w", bufs=1) as w_pool,
            tc.tile_pool(name="big", bufs=1) as big_pool,
            tc.tile_pool(name="work", bufs=3) as work_pool,
            tc.tile_pool(name="ps_main", bufs=3, space="PSUM") as ps_main,
            tc.tile_pool(name="ps_ctx", bufs=1, space="PSUM") as ps_ctx,
        ):
            # ---- load inputs ----
            # Order matters: wq/wk then the xt chunks gate the first Q/K
            # projection, which gates the whole pipeline. Everything else
            # streams behind them.
            xt_cm = tc.tile_pool(name="xt", bufs=8)
            xt_pool = xt_cm.__enter__()
            wq_t = w_pool.tile([128, 8, 256], BF16, tag="wq")
            wk_t = w_pool.tile([128, 8, 256], BF16, tag="wk")
            wv_t = w_pool.tile([128, 8, 256], BF16, tag="wv")
            wo_t = w_pool.tile([128, 2, D], BF16, tag="wo")
            padb_t = w_pool.tile([128, 16], F32, tag="padb")
            trimask_t = w_pool.tile([128, 2, 896], BF16, tag="trimask")
            # The first q-tile of x streams per-chunk so the Q projection's
            # accumulation steps pipeline with DMA arrival. The critical
            # startup prefix (wq/wk/xt q-tile 0) is split across the sync
            # and gpsimd DMA queues — a single queue tops out well below
            # HBM bandwidth. Everything the startup doesn't need is gated
            # behind the first K cast.
            xt = [
                xt_pool.tile([128, N], BF16, tag="xt", name=f"xt{e}")
                for e in range(8)
            ]
            # interleave the weight-chunk and x-chunk transfers so the first
            # Q/K accumulation steps start on the first ~0.3MB instead of
            # waiting for whole tensors
            # prefix split across three DMA queues (sync/vector/gpsimd) —
            # one queue tops out at ~160GB/s, a third of per-core HBM
            for e in range(8):
                nc.sync.dma_start(wq_t[:, e, :], wq_d[:, e * 256:(e + 1) * 256])
                eng = nc.sync if e % 2 == 0 else nc.scalar
                eng.dma_start(
                    xt[e][:, 0:512], xt_d[e * 128:(e + 1) * 128, 0:512]
                )
                nc.gpsimd.dma_start(wk_t[:, e, :], wk_d[:, e * 256:(e + 1) * 256])
            nc.sync.dma_start(wv_t[:].rearrange("p e m -> p (e m)"), wv_d[:])
            nc.sync.dma_start(padb_t[:], padb_d[:])
            nc.gpsimd.dma_start(
                trimask_t[:], trimask_d[:].rearrange("p (h u) -> p h u", h=2)
            )

            # Warm the ScalarE activation table (exp set) during the input
            # DMA window so the first real exp doesn't eat the ~2.7us load.
            warm_in = w_pool.tile([1, 8], F32, tag="warm_in")
            warm_out = w_pool.tile([1, 8], F32, tag="warm_out")
            nc.gpsimd.memset(warm_in[:], 0.0)
            nc.scalar.activation(warm_out[:], warm_in[:], EXP)

            # V' tile: [keys 128, key-block 16, head 4, 128]; col 64 <- ones,
            # cols 65-127 <- 0 (padding to 128 weight columns keeps FWL on for
            # the PV matmuls; PSUM rows 65-127 of ctx' are dead).
            v4 = big_pool.tile([128, NB, 4, 128], BF16, tag="v4")
            nc.vector.memset(v4[:, :, :, 65:128], 0.0)
            nc.sync.dma_start(
                v4[:, :, :, 64:65],
                ones_d[:].rearrange("p (b h o) -> p b h o", h=4, o=1),
            )

            qt_pair = [big_pool.tile([128, N], BF16, tag=f"qt{p}", name=f"qt{p}") for p in range(2)]
            kt_pair = [big_pool.tile([128, N], BF16, tag=f"kt{p}", name=f"kt{p}") for p in range(2)]
            ctx_pair = [big_pool.tile([128, N], BF16, tag=f"ctx{p}", name=f"ctx{p}") for p in range(2)]

            pt_cm = tc.tile_pool(name="pt", bufs=26)
            pt_pool = pt_cm.__enter__()

            # ---- projection tasks, riffled into the attention stream ----
            # QT/KT: [dk(128 = 2 heads), n] = (W.T chunk)^T @ xT
            def xt_slice(e, c0, c1):
                return xt[e][:, c0:c1]

            def emit_qk_proj(w_t, dst, pair, nq):
                ps = ps_main.tile([128, 2, 512], F32, tag="blk", name="blk")[:, 0, :]
                for e in range(8):
                    nc.tensor.matmul(
                        ps[:],
                        w_t[:, e, pair * 128:(pair + 1) * 128],
                        xt_slice(e, nq * 512, (nq + 1) * 512),
                        start=(e == 0),
                        stop=(e == 7),
                    )
                nc.vector.tensor_copy(dst[pair][:, nq * 512:(nq + 1) * 512], ps[:])

            def emit_qk_fused(pair, nq):
                # prologue: Q and K accumulate per x-chunk in lockstep so
                # both finish ~one matmul after the last x chunk arrives
                psq = ps_main.tile([128, 2, 512], F32, tag="blk", name="psq")[:, 0, :]
                psk = ps_main.tile([128, 2, 512], F32, tag="blk", name="psk")[:, 0, :]
                for e in range(8):
                    nc.tensor.matmul(
                        psq[:],
                        wq_t[:, e, pair * 128:(pair + 1) * 128],
                        xt_slice(e, nq * 512, (nq + 1) * 512),
                        start=(e == 0),
                        stop=(e == 7),
                    )
                    nc.tensor.matmul(
                        psk[:],
                        wk_t[:, e, pair * 128:(pair + 1) * 128],
                        xt_slice(e, nq * 512, (nq + 1) * 512),
                        start=(e == 0),
                        stop=(e == 7),
                    )
                nc.vector.tensor_copy(
                    kt_pair[pair][:, nq * 512:(nq + 1) * 512], psk[:]
                )
                nc.vector.tensor_copy(
                    qt_pair[pair][:, nq * 512:(nq + 1) * 512], psq[:]
                )

            # V natural: [n-block, 4*64] = xT-chunk^T @ WvT-chunk
            def emit_v_proj(nb):
                ps = ps_main.tile([128, 2, 512], F32, tag="blk", name="blk")[:, 0, 0:256]
                for e in range(8):
                    nc.tensor.matmul(
                        ps[:],
                        xt_slice(e, nb * 128, (nb + 1) * 128),
                        wv_t[:, e, :],
                        start=(e == 0),
                        stop=(e == 7),
                    )
                nc.vector.tensor_copy(
                    v4[:, nb, :, 0:64],
                    ps[:].rearrange("p (h d) -> p h d", h=4),
                )

            # Unit order: q0, q2, q3, q1. Starting on q0 keeps the cold-clock
            # prologue to two projection groups; the exp-heavy q2/q3 stretch
            # sits mid-stream where V-block/outproj filler is plentiful; the
            # small-PV q1 tail keeps the post-last-exp drain short.
            # Each projection task carries a (unit, slot) deadline: its S^T
            # needs Q at slot 0 and K tile t by slot 4t; the riffled PV of
            # unit u-1 needs V block nb by slot nb of unit u. Tasks drain
            # lazily at ~1 per 2 chunk slots so filler PE work carries
            # forward into the exp-heavy units; deadlines force correctness.
            qt_order = [0, 2, 3, 1]
            units = [
                (pair, qt, min(4 * qt + 4, kb_max))
                for qt in qt_order
                for pair in range(2)
            ]

            def t_q(pair, r):
                return lambda: emit_qk_proj(wq_t, qt_pair, pair, r)

            def t_k(pair, r):
                return lambda: emit_qk_proj(wk_t, kt_pair, pair, r)

            def t_v(nb):
                return lambda: emit_v_proj(nb)

            tasks = []  # (unit, slot, thunk)
            k_done = [set(), set()]
            v_done = set()
            for ui, (pair, qt, nchunks) in enumerate(units):
                if ui == 0:
                    # fused Q+K prologue for the very first unit
                    k_done[pair].add(qt)
                    tasks.append((0, 0, lambda p=pair, r=qt: emit_qk_fused(p, r)))
                else:
                    tasks.append((ui, 0, t_q(pair, qt)))
                for t in range(qt + 1):
                    if t not in k_done[pair]:
                        k_done[pair].add(t)
                        tasks.append((ui, 4 * t, t_k(pair, t)))
                if ui + 1 < len(units):
                    # V blocks consumed by this unit's PV, riffled in unit+1
                    for nb in range(nchunks):
                        if nb not in v_done:
                            v_done.add(nb)
                            tasks.append((ui + 1, nb, t_v(nb)))
            tasks.sort(key=lambda e: (e[0], e[1]))
            proj_fifo = list(tasks)
            filler_q = []  # deadline-free PE filler (outproj nb-blocks)

            def drain_due(ui, j):
                while proj_fifo and (proj_fifo[0][0], proj_fifo[0][1]) <= (ui, j):
                    proj_fifo.pop(0)[2]()

            def drain_lazy():
                if proj_fifo:
                    proj_fifo.pop(0)[2]()
                elif filler_q:
                    filler_q.pop(0)()

            # ---- attention, head pairs interleaved ----
            # A unit is (head-pair, q-tile). The two heads' S^T matmuls sit
            # at base partitions 0 / 64 (row groups 0-63 / 64-127), so they
            # execute concurrently on the PE and their weight loads overlap
            # the other head's matmul — no LDW bubble, HAM stays warm.
            # PV matmuls run one unit behind their exps so the in-order PE
            # never drains waiting on ScalarE.
            def emit_normalize(pair, hh, qt, ctx_ps):
                # the craw copy doubles as the PSUM release: it is the only
                # reader of ctx_ps rows 0-63, so the next unit's PV (same
                # single-buffered bank) can start as soon as it completes
                # instead of waiting for the whole normalize chain
                hp = slice(64 * hh, 64 * hh + 64)
                craw = work_pool.tile([64, 512], F32, tag="craw", name="craw")
                nc.vector.tensor_copy(craw[:], ctx_ps[0:64, :])
                rden = work_pool.tile([1, 512], F32, tag="rden", name="rden")
                nc.scalar.copy(rden[:], ctx_ps[64:65, :])
                rrec = work_pool.tile([1, 512], F32, tag="rrec", name="rrec")
                nc.vector.reciprocal_approx_fast(rrec[:], rden[:])
                rbr = work_pool.tile([64, 512], F32, tag="rbr", name="rbr")
                nc.gpsimd.partition_broadcast(rbr[:], rrec[:])
                nc.vector.tensor_mul(
                    ctx_pair[pair][hp, qt * 512:(qt + 1) * 512],
                    craw[0:64, :],
                    rbr[:],
                )

            def emit_st_exp(ui, pair, qt, nchunks, prev, self_ctx2=None):
                """S^T + mask + exp for both heads, with the previous unit's
                PV matmuls riffled in (they are long-ready and fill the PE
                slots where S^T would stall on the exp pipeline). Returns
                PV descriptors."""
                if prev is None:
                    ppv = []
                else:
                    ppair, pqt, pn, ppv, pctx2 = prev

                def rif(k):
                    # emit previous-unit PV chunks up to index k
                    while ppv and ppv[0][0] <= k:
                        jj, ptt, poff = ppv.pop(0)
                        for hh in range(2):
                            nc.tensor.matmul(
                                pctx2[hh][:, poff:],
                                v4[:, jj, 2 * ppair + hh, :],
                                ptt[:, hh, poff:],
                                start=(jj == 0),
                                stop=(jj == pn - 1),
                                skip_group_check=True,
                            )

                pv = []
                last_units = ui >= len(units) - 2
                for j in range(nchunks):
                    drain_due(ui, j)
                    rif(j)
                    # drain filler every other slot; every slot near the end
                    # so no PE work is left to trail the last exps
                    if last_units or j % 2 == 1:
                        drain_lazy()
                    d = j - 4 * qt
                    # exact-causal column trim (keep matmul N >= 256)
                    off = 128 * d if d >= 1 else 0
                    st_ps = ps_main.tile([128, 2, 512], F32, tag="blk", name="blk")
                    for hh in range(2):
                        hp = slice(64 * hh, 64 * hh + 64)
                        nc.tensor.matmul(
                            st_ps[:, hh, off:],
                            kt_pair[pair][hp, j * 128:(j + 1) * 128],
                            qt_pair[pair][hp, qt * 512 + off:(qt + 1) * 512],
                            start=True,
                            stop=True,
                        )
                    pt_t = pt_pool.tile([128, 2, 512], BF16, tag="pt")
                    kw = {}
                    if j >= jpad_min:  # per-key pad bias (same for both heads)
                        kw["bias"] = padb_t[:, j:j + 1]
                    nc.scalar.activation(
                        pt_t[:, :, off:], st_ps[:, :, off:], EXP, scale=SCALE, **kw
                    )
                    if d >= 0:
                        # causal mask as a post-exp 0/1 multiply; per-head 2D
                        # slices keep the DVE in its fast dense mode (a 3D
                        # strided AP drops it to 1x). With off = 128*d the
                        # masked triangle lies entirely in cols [off, off+128)
                        u0 = 384 - 128 * d + off
                        w = min(128, 512 - off)
                        for hh in range(2):
                            nc.vector.tensor_mul(
                                pt_t[:, hh, off:off + w],
                                pt_t[:, hh, off:off + w],
                                trimask_t[:, hh, u0:u0 + w],
                            )
                    pv.append((j, pt_t, off))
                    if self_ctx2 is not None and len(pv) >= 2:
                        # final unit: riffle its own PV one chunk behind the
                        # exps so the tail isn't a serial PV stream after
                        # the last exp
                        jj, ptt, poff = pv.pop(0)
                        for hh in range(2):
                            nc.tensor.matmul(
                                self_ctx2[hh][:, poff:],
                                v4[:, jj, 2 * pair + hh, :],
                                ptt[:, hh, poff:],
                                start=(jj == 0),
                                stop=(jj == nchunks - 1),
                                skip_group_check=True,
                            )
                rif(10 ** 9)
                return pv

            def emit_pv(pair, qt, nchunks, pv, ctx2):
                for j, pt_t, off in pv:
                    for hh in range(2):
                        nc.tensor.matmul(
                            ctx2[hh][:, off:],
                            v4[:, j, 2 * pair + hh, :],
                            pt_t[:, hh, off:],
                            start=(j == 0),
                            stop=(j == nchunks - 1),
                            skip_group_check=True,
                        )

            done_norms = {q: 0 for q in range(NQT)}
            d_emitted = set()

            def emit_outproj_nb(nb, split):
                # output projection for one 128-token block; `split` sends
                # the fc=1 PSUM->SBUF cast to ScalarE (tail drain, when
                # ScalarE has gone idle) instead of DVE.
                osb = work_pool.tile([128, D], BF16, tag="osb", name="osb")
                for fc in range(2):
                    ps = ps_main.tile(
                        [128, 2, 512], F32, tag="blk", name="blk"
                    )[:, 0, :]
                    for pr2 in range(2):
                        nc.tensor.matmul(
                            ps[:],
                            ctx_pair[pr2][:, nb * 128:(nb + 1) * 128],
                            wo_t[:, pr2, fc * 512:(fc + 1) * 512],
                            start=(pr2 == 0),
                            stop=(pr2 == 1),
                        )
                    if split and fc == 1:
                        nc.scalar.copy(osb[:, fc * 512:(fc + 1) * 512], ps[:])
                    else:
                        nc.vector.tensor_copy(
                            osb[:, fc * 512:(fc + 1) * 512], ps[:]
                        )
                if split:
                    # tail: split the write across two DMA queues (one queue
                    # tops out at ~140GB/s; gpsimd's queue is free by now)
                    nc.sync.dma_start(
                        out_d[nb * 128:(nb + 1) * 128, 0:512], osb[:, 0:512]
                    )
                    nc.gpsimd.dma_start(
                        out_d[nb * 128:(nb + 1) * 128, 512:D], osb[:, 512:D]
                    )
                else:
                    nc.sync.dma_start(out_d[nb * 128:(nb + 1) * 128, :], osb[:])

            def pop_norm(tail=False):
                npair, nqt, nctx2 = norm_q.pop(0)
                for hh in range(2):
                    emit_normalize(npair, hh, nqt, nctx2[hh])
                done_norms[nqt] += 1
                if done_norms[nqt] == 2 and nqt not in d_emitted:
                    d_emitted.add(nqt)
                    for nb in range(4 * nqt, 4 * nqt + 4):
                        filler_q.append(
                            lambda nb=nb: emit_outproj_nb(nb, tail)
                        )

            prev_pv = None  # (pair, qt, nchunks, pv_descs, ctx2)
            norm_q = []  # normalize one unit behind the PV
            for ui, (pair, qt, nchunks) in enumerate(units):
                if ui == 1:
                    # bulk xt/wout DMAs, gated on the first K cast so they
                    # don't steal HBM bandwidth from the startup prefix. The
                    # gate binds via a WAW hazard: a tiny copy that reads
                    # kt_pair (produced by the K cast) and writes into the
                    # DMA's destination region, so the scheduler cannot
                    # hoist the DMA ahead of it.
                    for e in range(8):
                        nc.gpsimd.tensor_copy(
                            xt[e][:, 512:516], kt_pair[0][:, 0:4]
                        )
                        nc.gpsimd.dma_start(
                            xt[e][:, 512:N], xt_d[e * 128:(e + 1) * 128, 512:N]
                        )
                    nc.gpsimd.tensor_copy(wo_t[:, 0, 0:4], kt_pair[0][:, 0:4])
                    nc.gpsimd.dma_start(
                        wo_t[:].rearrange("p c m -> p (c m)"), wout_d[:]
                    )
                pv = emit_st_exp(ui, pair, qt, nchunks, prev_pv)
                # normalize runs ONE unit behind its PV (which was just
                # riffled into this unit): popping here keeps the single
                # ctx PSUM buffer per head from serializing the next PV
                # against a 2-generation-old reader, and unlocks outproj
                # filler a unit earlier.
                if prev_pv is not None:
                    ppair, pqt, pn, ppv, pctx2 = prev_pv
                    norm_q.append((ppair, pqt, pctx2))
                if norm_q:
                    pop_norm()
                ctx2 = [
                    ps_ctx.tile([128, 512], F32, tag=f"ctx{hh}", name=f"ctx{hh}")
                    for hh in range(2)
                ]
                prev_pv = (pair, qt, nchunks, pv, ctx2)
            # flush leftover filler (outproj blocks of earlier q-tiles)
            # BEFORE the final PV/normalize chain so the PE stays busy —
            # an idle sliver here re-throttles the clock for the whole tail
            while filler_q:
                filler_q.pop(0)()
            ppair, pqt, pn, ppv, pctx2 = prev_pv
            emit_pv(ppair, pqt, pn, ppv, pctx2)
            norm_q.append((ppair, pqt, pctx2))
            while norm_q:
                pop_norm(tail=True)
            while filler_q:
                filler_q.pop(0)()

            pt_cm.__exit__(None, None, None)
            xt_cm.__exit__(None, None, None)

    nc.compile()
    return nc


_PROGRAM_CACHE = {}


def kernel(x, attention_mask, W_Q, W_K, W_V, W_out, b_out):
    global LAST_RESULTS
    from concourse.bass_utils import run_bass_kernel_spmd

    x = np.ascontiguousarray(x, dtype=np.float32)
    attention_mask = np.asarray(attention_mask)
    lengths = attention_mask.astype(np.int64).sum(axis=1)
    kb_max = int(math.ceil(lengths.max() / KBLK))
    jpad_min = int(lengths.min() // KBLK)

    key = (kb_max, jpad_min)
    if key not in _PROGRAM_CACHE:
        _PROGRAM_CACHE[key] = _build_program(kb_max, jpad_min)
    nc = _PROGRAM_CACHE[key]

    # host-side input prep (matmul operands pre-cast to bf16)
    import ml_dtypes
    BF = ml_dtypes.bfloat16
    xT = [np.ascontiguousarray(x[b].T.astype(BF)) for b in range(B)]
    wqT = np.ascontiguousarray(np.asarray(W_Q, dtype=np.float32).T.astype(BF))
    wkT = np.ascontiguousarray(np.asarray(W_K, dtype=np.float32).T.astype(BF))
    wvT = np.ascontiguousarray(np.asarray(W_V, dtype=np.float32).T.astype(BF))
    woT = np.ascontiguousarray(np.asarray(W_out, dtype=np.float32).T.astype(BF))
    # padbias[p, j] = 0 if key j*128+p is real else -30000
    padb = [
        np.ascontiguousarray(
            np.where(attention_mask[b].reshape(16, 128).T != 0, 0.0, NEGB)
        ).astype(np.float32)
        for b in range(B)
    ]
    # trimask[p, u] = 0 if u < p + 384 else 1; slice [384-128d : 896-128d]
    # gives the causal 0/1 mask for a diagonal block with offset 128d,
    # duplicated along the 2-head axis -> [128, 2*896].
    pp = np.arange(128)[:, None]
    uu = np.arange(896)[None, :]
    trimask1 = np.where(uu < pp + 384, 0.0, 1.0)
    trimask = np.ascontiguousarray(
        np.broadcast_to(trimask1[:, None, :], (128, 2, 896)).reshape(128, 1792)
    ).astype(BF)
    ones65 = np.ones((128, 64), dtype=BF)

    def pack_w(w):  # [1024, 256] -> on-chip [128, 8, 256] layout, flattened
        return np.ascontiguousarray(
            w.reshape(8, 128, 256).transpose(1, 0, 2).reshape(128, 2048)
        )

    def pack_wo(w):  # [256, 1024] -> on-chip [128, 2, 1024] layout, flattened
        return np.ascontiguousarray(
            w.reshape(2, 128, 1024).transpose(1, 0, 2).reshape(128, 2048)
        )

    in_maps = []
    for c in range(NCORES):
        b, g = divmod(c, 4)
        sl = slice(g * 256, (g + 1) * 256)
        in_maps.append(
            {
                "xt": xT[b],
                "wq": pack_w(wqT[:, sl]),
                "wk": pack_w(wkT[:, sl]),
                "wv": pack_w(wvT[:, sl]),
                "wout": pack_wo(woT[sl, :]),
                "padbias": padb[b],
                "trimask": trimask,
                "ones65": ones65,
            }
        )

    trace = bool(int(os.environ.get("KERNEL_TRACE", "0")))
    ncores_run = int(os.environ.get("KERNEL_NCORES", str(NCORES)))
    res = run_bass_kernel_spmd(
        nc,
        in_maps[:ncores_run],
        core_ids=list(range(ncores_run)),
        trace=trace,
        trace_cores=list(range(ncores_run)) if trace else None,
    )
    LAST_RESULTS = res

    out = np.zeros((B, N, D), dtype=np.float32)
    for c in range(len(res.results)):
        out[c // 4] += res.results[c]["out"].astype(np.float32)
    out += np.asarray(b_out, dtype=np.float32)[None, None, :]
    return out



# revision 91
# speedup vs baseline: 1.1455x; 1.0229x over previous
"""Trainium2 Bass kernel for causal+padded multi-head attention.

Problem: B=2, N=2048, D=1024, H=16 heads (DK=64), fp32 I/O.
  out = softmax(mask(x Wq^T (x Wk^T)^T) / sqrt(DK)) (x Wv^T) Wout^T + b_out

Sharding (8 cores): core c handles batch b=c//4 and heads [4*(c%4), 4*(c%4)+4).
Each core computes a partial output [N, D] (its 4 heads' contribution through
the output projection) in bf16; the host sums the 4 partials per batch and
adds b_out.

Design (per core) — a single software-pipelined stream; the projections are
NOT a separate phase but are riffled into the attention schedule as PE
filler so ScalarE (exp) ramps up ~15us in and the PE never idles long
enough to re-throttle the HAM clock gate to 1.2 GHz:

  xT   [1024, 2048] (host-pretransposed x[b]); QT/KT transposed [dk, n] as
       head-pair tiles [128, 2048]; V natural as [128(keys), 16, 4, 128]
       where col 64 is a ones column (so P@V' also yields the softmax
       denominator) and cols 65-127 are zero padding (keeps NumWeights==128
       so FWL hides the PV weight loads).
  Units are (head-pair, q-tile 512), processed in q-tile order 0,2,3,1:
       q0 first keeps the cold-clock prologue to one fused Q+K group; the
       exp-heavy q2/q3 stretch sits mid-stream where V-block/outproj filler
       is plentiful; the small-PV q1 tail keeps the post-last-exp drain
       short.
  S^T  per (unit, key-block 128) as [128, 2, 512] PSUM: matmul(lhsT=KT
       slice [64,128], rhs=QT slice [64,512]); the two heads sit at base
       partitions 0/64 so their matmuls row-tile and run concurrently.
       One exp(0.125*s + padbias) on ScalarE (bias = per-key pad mask)
       writes P^T to SBUF bf16; causal masking is a post-exp 0/1 multiply
       on DVE (per-head 2D slices keep the fast dense mode).
  PV   [128, 512] PSUM accumulated over key blocks, riffled one unit behind
       the exps between the next unit's S^T matmuls.
  Norm (one unit behind PV): DVE copy of ctx rows (doubles as the PSUM
       release), ScalarE copy of the denominator row, reciprocal_approx_fast
       (DVE; input must be a base-partition-0 AP), GpSimd partition
       broadcast, one DVE multiply -> ctx bf16.
  Outproj per 128-token block, queued as deadline-free PE filler and
       drained into exp-heavy units; PSUM->SBUF casts on DVE (tail: split
       DVE/ScalarE), output written bf16, split across two DMA queues in
       the tail (one queue tops out at ~140GB/s).
  DMA  the critical prefix (wq/wk interleaved with xt q-tile 0) is split
       across the sync/scalar/gpsimd queues; all weights arrive host-packed
       in the on-chip layout so every transfer is contiguous; the bulk of
       xt and wout are gated behind the first K cast (WAW hazard via a tiny
       copy) so they don't steal HBM bandwidth from the prefix; the ScalarE
       exp table is prefetched with a dummy activation at t=0.

All matmul operands are bf16 (pre-rounded on host for the inputs; on-device
casts for intermediates); accumulation is fp32 in PSUM, and the softmax /
normalization arithmetic is fp32.
"""

import math
import os

import numpy as np

B, N, D, H = 2, 2048, 1024, 16
DK = D // H  # 64
NCORES = 8
HEADS_PER_CORE = 4
QTILE = 512
KBLK = 128
NEG = -30000.0
NEGB = -3750.0  # pad bias applied after the 0.125 scale inside exp
SCALE = 1.0 / math.sqrt(float(DK))  # 0.125

# Set by run() when tracing is enabled (test.py reads this).
LAST_RESULTS = None


def _build_program(kb_max: int, jpad_min: int):
    import concourse.tile as tile
    from concourse import bacc, mybir

    F32 = mybir.dt.float32
    F32R = mybir.dt.float32r
    BF16 = mybir.dt.bfloat16
    EXP = mybir.ActivationFunctionType.Exp
    ADD = mybir.AluOpType.add

    nc = bacc.Bacc(None)

    # weights arrive pre-arranged on the host into the on-chip layout so
    # every DMA is a contiguous [128, x] transfer (strided gathers stall the
    # critical first-projection path)
    xt_d = nc.dram_tensor("xt", [D, N], BF16, kind="ExternalInput")
    wq_d = nc.dram_tensor("wq", [128, 2048], BF16, kind="ExternalInput")
    wk_d = nc.dram_tensor("wk", [128, 2048], BF16, kind="ExternalInput")
    wv_d = nc.dram_tensor("wv", [128, 2048], BF16, kind="ExternalInput")
    wout_d = nc.dram_tensor("wout", [128, 2048], BF16, kind="ExternalInput")
    padb_d = nc.dram_tensor("padbias", [128, 16], F32, kind="ExternalInput")
    trimask_d = nc.dram_tensor("trimask", [128, 1792], BF16, kind="ExternalInput")
    ones_d = nc.dram_tensor("ones65", [128, 64], BF16, kind="ExternalInput")
    out_d = nc.dram_tensor("out", [N, D], BF16, kind="ExternalOutput")

    NB = N // KBLK  # 16 key/row blocks
    NQT = N // QTILE  # 4 q tiles

    with tile.TileContext(nc) as tc:
        with (
            tc.tile_pool(name="w", bufs=1) as w_pool,
            tc.tile_pool(name="big", bufs=1) as big_pool,
            tc.tile_pool(name="work", bufs=3) as work_pool,
            tc.tile_pool(name="ps_main", bufs=3, space="PSUM") as ps_main,
            tc.tile_pool(name="ps_ctx", bufs=1, space="PSUM") as ps_ctx,
        ):
            # ---- load inputs ----
            # Order matters: wq/wk then the xt chunks gate the first Q/K
            # projection, which gates the whole pipeline. Everything else
            # streams behind them.
            xt_cm = tc.tile_pool(name="xt", bufs=8)
            xt_pool = xt_cm.__enter__()
            wq_t = w_pool.tile([128, 8, 256], BF16, tag="wq")
            wk_t = w_pool.tile([128, 8, 256], BF16, tag="wk")
            wv_t = w_pool.tile([128, 8, 256], BF16, tag="wv")
            wo_t = w_pool.tile([128, 2, D], BF16, tag="wo")
            padb_t = w_pool.tile([128, 16], F32, tag="padb")
            trimask_t = w_pool.tile([128, 2, 896], BF16, tag="trimask")
            # The first q-tile of x streams per-chunk so the Q projection's
            # accumulation steps pipeline with DMA arrival. The critical
            # startup prefix (wq/wk/xt q-tile 0) is split across the sync
            # and gpsimd DMA queues — a single queue tops out well below
            # HBM bandwidth. Everything the startup doesn't need is gated
            # behind the first K cast.
            xt = [
                xt_pool.tile([128, N], BF16, tag="xt", name=f"xt{e}")
                for e in range(8)
            ]
            # interleave the weight-chunk and x-chunk transfers so the first
            # Q/K accumulation steps start on the first ~0.3MB instead of
            # waiting for whole tensors
            # prefix split across three DMA queues (sync/vector/gpsimd) —
            # one queue tops out at ~160GB/s, a third of per-core HBM
            for e in range(8):
                nc.sync.dma_start(wq_t[:, e, :], wq_d[:, e * 256:(e + 1) * 256])
                eng = nc.sync if e % 2 == 0 else nc.scalar
                eng.dma_start(
                    xt[e][:, 0:512], xt_d[e * 128:(e + 1) * 128, 0:512]
                )
                nc.gpsimd.dma_start(wk_t[:, e, :], wk_d[:, e * 256:(e + 1) * 256])
            nc.sync.dma_start(wv_t[:].rearrange("p e m -> p (e m)"), wv_d[:])
            nc.sync.dma_start(padb_t[:], padb_d[:])
            nc.gpsimd.dma_start(
                trimask_t[:], trimask_d[:].rearrange("p (h u) -> p h u", h=2)
            )

            # Warm the ScalarE activation table (exp set) during the input
            # DMA window so the first real exp doesn't eat the ~2.7us load.
            warm_in = w_pool.tile([1, 8], F32, tag="warm_in")
            warm_out = w_pool.tile([1, 8], F32, tag="warm_out")
            nc.gpsimd.memset(warm_in[:], 0.0)
            nc.scalar.activation(warm_out[:], warm_in[:], EXP)

            # Pre-heat the PE during the same window: ~10us of back-to-back
            # dummy matmuls keep the HAM activity monitor's busy window
            # satisfied, so the prologue projections start at 2.4GHz instead
            # of the cold 1.2GHz (the gate needs ~3.4us of sustained
            # activity to open and re-closes after ~3.4us idle).
            warm_w = w_pool.tile([128, 16], BF16, tag="warm_w")
            warm_big = w_pool.tile([128, 512], BF16, tag="warm_big")
            nc.vector.memset(warm_w[:], 0.0)
            nc.vector.memset(warm_big[:], 0.0)
            ps_warm = ps_main.tile([128, 2, 512], F32, tag="blk", name="ps_warm")
            for i in range(36):
                nc.tensor.matmul(
                    ps_warm[0:16, 0, :],
                    warm_w[:],
                    warm_big[:],
                    start=(i == 0),
                    stop=(i == 35),
                )

            # V' tile: [keys 128, key-block 16, head 4, 128]; col 64 <- ones,
            # cols 65-127 <- 0 (padding to 128 weight columns keeps FWL on for
            # the PV matmuls; PSUM rows 65-127 of ctx' are dead).
            v4 = big_pool.tile([128, NB, 4, 128], BF16, tag="v4")
            nc.vector.memset(v4[:, :, :, 65:128], 0.0)
            nc.sync.dma_start(
                v4[:, :, :, 64:65],
                ones_d[:].rearrange("p (b h o) -> p b h o", h=4, o=1),
            )

            qt_pair = [big_pool.tile([128, N], BF16, tag=f"qt{p}", name=f"qt{p}") for p in range(2)]
            kt_pair = [big_pool.tile([128, N], BF16, tag=f"kt{p}", name=f"kt{p}") for p in range(2)]
            ctx_pair = [big_pool.tile([128, N], BF16, tag=f"ctx{p}", name=f"ctx{p}") for p in range(2)]

            pt_cm = tc.tile_pool(name="pt", bufs=26)
            pt_pool = pt_cm.__enter__()

            # ---- projection tasks, riffled into the attention stream ----
            # QT/KT: [dk(128 = 2 heads), n] = (W.T chunk)^T @ xT
            def xt_slice(e, c0, c1):
                return xt[e][:, c0:c1]

            def emit_qk_proj(w_t, dst, pair, nq):
                ps = ps_main.tile([128, 2, 512], F32, tag="blk", name="blk")[:, 0, :]
                for e in range(8):
                    nc.tensor.matmul(
                        ps[:],
                        w_t[:, e, pair * 128:(pair + 1) * 128],
                        xt_slice(e, nq * 512, (nq + 1) * 512),
                        start=(e == 0),
                        stop=(e == 7),
                    )
                nc.vector.tensor_copy(dst[pair][:, nq * 512:(nq + 1) * 512], ps[:])

            def emit_qk_fused(pair, nq):
                # prologue: Q and K accumulate per x-chunk in lockstep so
                # both finish ~one matmul after the last x chunk arrives
                psq = ps_main.tile([128, 2, 512], F32, tag="blk", name="psq")[:, 0, :]
                psk = ps_main.tile([128, 2, 512], F32, tag="blk", name="psk")[:, 0, :]
                for e in range(8):
                    nc.tensor.matmul(
                        psq[:],
                        wq_t[:, e, pair * 128:(pair + 1) * 128],
                        xt_slice(e, nq * 512, (nq + 1) * 512),
                        start=(e == 0),
                        stop=(e == 7),
                    )
                    nc.tensor.matmul(
                        psk[:],
                        wk_t[:, e, pair * 128:(pair + 1) * 128],
                        xt_slice(e, nq * 512, (nq + 1) * 512),
                        start=(e == 0),
                        stop=(e == 7),
                    )
                nc.vector.tensor_copy(
                    kt_pair[pair][:, nq * 512:(nq + 1) * 512], psk[:]
                )
                nc.vector.tensor_copy(
                    qt_pair[pair][:, nq * 512:(nq + 1) * 512], psq[:]
                )

            # V natural: [n-block, 4*64] = xT-chunk^T @ WvT-chunk
            def emit_v_proj(nb):
                ps = ps_main.tile([128, 2, 512], F32, tag="blk", name="blk")[:, 0, 0:256]
                for e in range(8):
                    nc.tensor.matmul(
                        ps[:],
                        xt_slice(e, nb * 128, (nb + 1) * 128),
                        wv_t[:, e, :],
                        start=(e == 0),
                        stop=(e == 7),
                    )
                nc.vector.tensor_copy(
                    v4[:, nb, :, 0:64],
                    ps[:].rearrange("p (h d) -> p h d", h=4),
                )

            # Unit order: q0, q2, q3, q1. Starting on q0 keeps the cold-clock
            # prologue to two projection groups; the exp-heavy q2/q3 stretch
            # sits mid-stream where V-block/outproj filler is plentiful; the
            # small-PV q1 tail keeps the post-last-exp drain short.
            # Each projection task carries a (unit, slot) deadline: its S^T
            # needs Q at slot 0 and K tile t by slot 4t; the riffled PV of
            # unit u-1 needs V block nb by slot nb of unit u. Tasks drain
            # lazily at ~1 per 2 chunk slots so filler PE work carries
            # forward into the exp-heavy units; deadlines force correctness.
            qt_order = [0, 2, 3, 1]
            units = [
                (pair, qt, min(4 * qt + 4, kb_max))
                for qt in qt_order
                for pair in range(2)
            ]

            def t_q(pair, r):
                return lambda: emit_qk_proj(wq_t, qt_pair, pair, r)

            def t_k(pair, r):
                return lambda: emit_qk_proj(wk_t, kt_pair, pair, r)

            def t_v(nb):
                return lambda: emit_v_proj(nb)

            tasks = []  # (unit, slot, thunk)
            k_done = [set(), set()]
            v_done = set()
            for ui, (pair, qt, nchunks) in enumerate(units):
                if ui == 0:
                    # fused Q+K prologue for the very first unit
                    k_done[pair].add(qt)
                    tasks.append((0, 0, lambda p=pair, r=qt: emit_qk_fused(p, r)))
                else:
                    tasks.append((ui, 0, t_q(pair, qt)))
                for t in range(qt + 1):
                    if t not in k_done[pair]:
                        k_done[pair].add(t)
                        tasks.append((ui, 4 * t, t_k(pair, t)))
                if ui + 1 < len(units):
                    # V blocks consumed by this unit's PV, riffled in unit+1
                    for nb in range(nchunks):
                        if nb not in v_done:
                            v_done.add(nb)
                            tasks.append((ui + 1, nb, t_v(nb)))
            tasks.sort(key=lambda e: (e[0], e[1]))
            proj_fifo = list(tasks)
            filler_q = []  # deadline-free PE filler (outproj nb-blocks)

            def drain_due(ui, j):
                while proj_fifo and (proj_fifo[0][0], proj_fifo[0][1]) <= (ui, j):
                    proj_fifo.pop(0)[2]()

            def drain_lazy():
                if proj_fifo:
                    proj_fifo.pop(0)[2]()
                elif filler_q:
                    filler_q.pop(0)()

            # ---- attention, head pairs interleaved ----
            # A unit is (head-pair, q-tile). The two heads' S^T matmuls sit
            # at base partitions 0 / 64 (row groups 0-63 / 64-127), so they
            # execute concurrently on the PE and their weight loads overlap
            # the other head's matmul — no LDW bubble, HAM stays warm.
            # PV matmuls run one unit behind their exps so the in-order PE
            # never drains waiting on ScalarE.
            def emit_normalize(pair, hh, qt, ctx_ps):
                # the craw copy doubles as the PSUM release: it is the only
                # reader of ctx_ps rows 0-63, so the next unit's PV (same
                # single-buffered bank) can start as soon as it completes
                # instead of waiting for the whole normalize chain
                hp = slice(64 * hh, 64 * hh + 64)
                craw = work_pool.tile([64, 512], F32, tag="craw", name="craw")
                nc.vector.tensor_copy(craw[:], ctx_ps[0:64, :])
                rden = work_pool.tile([1, 512], F32, tag="rden", name="rden")
                nc.scalar.copy(rden[:], ctx_ps[64:65, :])
                rrec = work_pool.tile([1, 512], F32, tag="rrec", name="rrec")
                nc.vector.reciprocal_approx_fast(rrec[:], rden[:])
                rbr = work_pool.tile([64, 512], F32, tag="rbr", name="rbr")
                nc.gpsimd.partition_broadcast(rbr[:], rrec[:])
                nc.vector.tensor_mul(
                    ctx_pair[pair][hp, qt * 512:(qt + 1) * 512],
                    craw[0:64, :],
                    rbr[:],
                )

            def emit_st_exp(ui, pair, qt, nchunks, prev, self_ctx2=None):
                """S^T + mask + exp for both heads, with the previous unit's
                PV matmuls riffled in (they are long-ready and fill the PE
                slots where S^T would stall on the exp pipeline). Returns
                PV descriptors."""
                if prev is None:
                    ppv = []
                else:
                    ppair, pqt, pn, ppv, pctx2 = prev

                def rif(k):
                    # emit previous-unit PV chunks up to index k
                    while ppv and ppv[0][0] <= k:
                        jj, ptt, poff = ppv.pop(0)
                        for hh in range(2):
                            nc.tensor.matmul(
                                pctx2[hh][:, poff:],
                                v4[:, jj, 2 * ppair + hh, :],
                                ptt[:, hh, poff:],
                                start=(jj == 0),
                                stop=(jj == pn - 1),
                                skip_group_check=True,
                            )

                pv = []
                last_units = ui >= len(units) - 2
                for j in range(nchunks):
                    drain_due(ui, j)
                    rif(j)
                    # drain filler every other slot; every slot near the end
                    # so no PE work is left to trail the last exps
                    if last_units or j % 2 == 1:
                        drain_lazy()
                    d = j - 4 * qt
                    # exact-causal column trim (keep matmul N >= 256)
                    off = 128 * d if d >= 1 else 0
                    st_ps = ps_main.tile([128, 2, 512], F32, tag="blk", name="blk")
                    for hh in range(2):
                        hp = slice(64 * hh, 64 * hh + 64)
                        nc.tensor.matmul(
                            st_ps[:, hh, off:],
                            kt_pair[pair][hp, j * 128:(j + 1) * 128],
                            qt_pair[pair][hp, qt * 512 + off:(qt + 1) * 512],
                            start=True,
                            stop=True,
                        )
                    pt_t = pt_pool.tile([128, 2, 512], BF16, tag="pt")
                    kw = {}
                    if j >= jpad_min:  # per-key pad bias (same for both heads)
                        kw["bias"] = padb_t[:, j:j + 1]
                    nc.scalar.activation(
                        pt_t[:, :, off:], st_ps[:, :, off:], EXP, scale=SCALE, **kw
                    )
                    if d >= 0:
                        # causal mask as a post-exp 0/1 multiply; per-head 2D
                        # slices keep the DVE in its fast dense mode (a 3D
                        # strided AP drops it to 1x). With off = 128*d the
                        # masked triangle lies entirely in cols [off, off+128)
                        u0 = 384 - 128 * d + off
                        w = min(128, 512 - off)
                        for hh in range(2):
                            nc.vector.tensor_mul(
                                pt_t[:, hh, off:off + w],
                                pt_t[:, hh, off:off + w],
                                trimask_t[:, hh, u0:u0 + w],
                            )
                    pv.append((j, pt_t, off))
                    if self_ctx2 is not None and len(pv) >= 2:
                        # final unit: riffle its own PV one chunk behind the
                        # exps so the tail isn't a serial PV stream after
                        # the last exp
                        jj, ptt, poff = pv.pop(0)
                        for hh in range(2):
                            nc.tensor.matmul(
                                self_ctx2[hh][:, poff:],
                                v4[:, jj, 2 * pair + hh, :],
                                ptt[:, hh, poff:],
                                start=(jj == 0),
                                stop=(jj == nchunks - 1),
                                skip_group_check=True,
                            )
                rif(10 ** 9)
                return pv

            def emit_pv(pair, qt, nchunks, pv, ctx2):
                for j, pt_t, off in pv:
                    for hh in range(2):
                        nc.tensor.matmul(
                            ctx2[hh][:, off:],
                            v4[:, j, 2 * pair + hh, :],
                            pt_t[:, hh, off:],
                            start=(j == 0),
                            stop=(j == nchunks - 1),
                            skip_group_check=True,
                        )

            done_norms = {q: 0 for q in range(NQT)}
            d_emitted = set()

            def emit_outproj_nb(nb, split):
                # output projection for one 128-token block; `split` sends
                # the fc=1 PSUM->SBUF cast to ScalarE (tail drain, when
                # ScalarE has gone idle) instead of DVE.
                osb = work_pool.tile([128, D], BF16, tag="osb", name="osb")
                for fc in range(2):
                    ps = ps_main.tile(
                        [128, 2, 512], F32, tag="blk", name="blk"
                    )[:, 0, :]
                    for pr2 in range(2):
                        nc.tensor.matmul(
                            ps[:],
                            ctx_pair[pr2][:, nb * 128:(nb + 1) * 128],
                            wo_t[:, pr2, fc * 512:(fc + 1) * 512],
                            start=(pr2 == 0),
                            stop=(pr2 == 1),
                        )
                    if split and fc == 1:
                        nc.scalar.copy(osb[:, fc * 512:(fc + 1) * 512], ps[:])
                    else:
                        nc.vector.tensor_copy(
                            osb[:, fc * 512:(fc + 1) * 512], ps[:]
                        )
                if split:
                    # tail: split the write across two DMA queues (one queue
                    # tops out at ~140GB/s; gpsimd's queue is free by now)
                    nc.sync.dma_start(
                        out_d[nb * 128:(nb + 1) * 128, 0:512], osb[:, 0:512]
                    )
                    nc.gpsimd.dma_start(
                        out_d[nb * 128:(nb + 1) * 128, 512:D], osb[:, 512:D]
                    )
                else:
                    nc.sync.dma_start(out_d[nb * 128:(nb + 1) * 128, :], osb[:])

            def pop_norm(tail=False):
                npair, nqt, nctx2 = norm_q.pop(0)
                for hh in range(2):
                    emit_normalize(npair, hh, nqt, nctx2[hh])
                done_norms[nqt] += 1
                if done_norms[nqt] == 2 and nqt not in d_emitted:
                    d_emitted.add(nqt)
                    for nb in range(4 * nqt, 4 * nqt + 4):
                        filler_q.append(
                            lambda nb=nb: emit_outproj_nb(nb, tail)
                        )

            prev_pv = None  # (pair, qt, nchunks, pv_descs, ctx2)
            norm_q = []  # normalize one unit behind the PV
            for ui, (pair, qt, nchunks) in enumerate(units):
                if ui == 1:
                    # bulk xt/wout DMAs, gated on the first K cast so they
                    # don't steal HBM bandwidth from the startup prefix. The
                    # gate binds via a WAW hazard: a tiny copy that reads
                    # kt_pair (produced by the K cast) and writes into the
                    # DMA's destination region, so the scheduler cannot
                    # hoist the DMA ahead of it.
                    for e in range(8):
                        nc.gpsimd.tensor_copy(
                            xt[e][:, 512:516], kt_pair[0][:, 0:4]
                        )
                        nc.gpsimd.dma_start(
                            xt[e][:, 512:N], xt_d[e * 128:(e + 1) * 128, 512:N]
                        )
                    nc.gpsimd.tensor_copy(wo_t[:, 0, 0:4], kt_pair[0][:, 0:4])
                    nc.gpsimd.dma_start(
                        wo_t[:].rearrange("p c m -> p (c m)"), wout_d[:]
                    )
                pv = emit_st_exp(ui, pair, qt, nchunks, prev_pv)
                # normalize runs ONE unit behind its PV (which was just
                # riffled into this unit): popping here keeps the single
                # ctx PSUM buffer per head from serializing the next PV
                # against a 2-generation-old reader, and unlocks outproj
                # filler a unit earlier.
                if prev_pv is not None:
                    ppair, pqt, pn, ppv, pctx2 = prev_pv
                    norm_q.append((ppair, pqt, pctx2))
                if norm_q:
                    pop_norm()
                ctx2 = [
                    ps_ctx.tile([128, 512], F32, tag=f"ctx{hh}", name=f"ctx{hh}")
                    for hh in range(2)
                ]
                prev_pv = (pair, qt, nchunks, pv, ctx2)
            # flush leftover filler (outproj blocks of earlier q-tiles)
            # BEFORE the final PV/normalize chain so the PE stays busy —
            # an idle sliver here re-throttles the clock for the whole tail
            while filler_q:
                filler_q.pop(0)()
            ppair, pqt, pn, ppv, pctx2 = prev_pv
            emit_pv(ppair, pqt, pn, ppv, pctx2)
            norm_q.append((ppair, pqt, pctx2))
            while norm_q:
                pop_norm(tail=True)
            while filler_q:
                filler_q.pop(0)()

            pt_cm.__exit__(None, None, None)
            xt_cm.__exit__(None, None, None)

    nc.compile()
    return nc


_PROGRAM_CACHE = {}


def kernel(x, attention_mask, W_Q, W_K, W_V, W_out, b_out):
    global LAST_RESULTS
    from concourse.bass_utils import run_bass_kernel_spmd

    x = np.ascontiguousarray(x, dtype=np.float32)
    attention_mask = np.asarray(attention_mask)
    lengths = attention_mask.astype(np.int64).sum(axis=1)
    kb_max = int(math.ceil(lengths.max() / KBLK))
    jpad_min = int(lengths.min() // KBLK)

    key = (kb_max, jpad_min)
    if key not in _PROGRAM_CACHE:
        _PROGRAM_CACHE[key] = _build_program(kb_max, jpad_min)
    nc = _PROGRAM_CACHE[key]

    # host-side input prep (matmul operands pre-cast to bf16)
    import ml_dtypes
    BF = ml_dtypes.bfloat16
    xT = [np.ascontiguousarray(x[b].T.astype(BF)) for b in range(B)]
    wqT = np.ascontiguousarray(np.asarray(W_Q, dtype=np.float32).T.astype(BF))
    wkT = np.ascontiguousarray(np.asarray(W_K, dtype=np.float32).T.astype(BF))
    wvT = np.ascontiguousarray(np.asarray(W_V, dtype=np.float32).T.astype(BF))
    woT = np.ascontiguousarray(np.asarray(W_out, dtype=np.float32).T.astype(BF))
    # padbias[p, j] = 0 if key j*128+p is real else -30000
    padb = [
        np.ascontiguousarray(
            np.where(attention_mask[b].reshape(16, 128).T != 0, 0.0, NEGB)
        ).astype(np.float32)
        for b in range(B)
    ]
    # trimask[p, u] = 0 if u < p + 384 else 1; slice [384-128d : 896-128d]
    # gives the causal 0/1 mask for a diagonal block with offset 128d,
    # duplicated along the 2-head axis -> [128, 2*896].
    pp = np.arange(128)[:, None]
    uu = np.arange(896)[None, :]
    trimask1 = np.where(uu < pp + 384, 0.0, 1.0)
    trimask = np.ascontiguousarray(
        np.broadcast_to(trimask1[:, None, :], (128, 2, 896)).reshape(128, 1792)
    ).astype(BF)
    ones65 = np.ones((128, 64), dtype=BF)

    def pack_w(w):  # [1024, 256] -> on-chip [128, 8, 256] layout, flattened
        return np.ascontiguousarray(
            w.reshape(8, 128, 256).transpose(1, 0, 2).reshape(128, 2048)
        )

    def pack_wo(w):  # [256, 1024] -> on-chip [128, 2, 1024] layout, flattened
        return np.ascontiguousarray(
            w.reshape(2, 128, 1024).transpose(1, 0, 2).reshape(128, 2048)
        )

    in_maps = []
    for c in range(NCORES):
        b, g = divmod(c, 4)
        sl = slice(g * 256, (g + 1) * 256)
        in_maps.append(
            {
                "xt": xT[b],
                "wq": pack_w(wqT[:, sl]),
                "wk": pack_w(wkT[:, sl]),
                "wv": pack_w(wvT[:, sl]),
                "wout": pack_wo(woT[sl, :]),
                "padbias": padb[b],
                "trimask": trimask,
                "ones65": ones65,
            }
        )

    trace = bool(int(os.environ.get("KERNEL_TRACE", "0")))
    ncores_run = int(os.environ.get("KERNEL_NCORES", str(NCORES)))
    res = run_bass_kernel_spmd(
        nc,
        in_maps[:ncores_run],
        core_ids=list(range(ncores_run)),
        trace=trace,
        trace_cores=list(range(ncores_run)) if trace else None,
    )
    LAST_RESULTS = res

    out = np.zeros((B, N, D), dtype=np.float32)
    for c in range(len(res.results)):
        out[c // 4] += res.results[c]["out"].astype(np.float32)
    out += np.asarray(b_out, dtype=np.float32)[None, None, :]
    return out

